# revision 1
# baseline (speedup 1.0000x reference)
"""v2: sparse expert-parallel MoE on 8 trn2 cores.

Core c computes: its expert's FFN on only the ~2100 tokens routed to it
(host-gathered, padded per token-range to 128-multiples), with routing
weights recomputed on device from the gathered activations; plus a 1/8
tensor-parallel slice of the shared expert over all tokens, gated by an
on-device sigmoid.

Output combine, token-major, one partial [TRANGE, H] per token range:
  1. partials zeroed via DMA (overlaps early compute)
  2. expert pass dma_scatter_add's its weighted rows by in-range token index
     (-1 pads are skipped by the ucode)
  3. shared pass accumulate-DMAs densely, ranges in order
  4. per-range ReduceScatter overlaps trailing shared compute
Host does: routing decision (indices only), gather, reassembly.

The per-range routed-token counts are derived from the actual inputs and
baked into the compiled program (compilation happens inside kernel(), after
routing), so capacity always fits; the build is cached keyed on the counts.
"""

import numpy as np
from contextlib import ExitStack

import concourse.bass as bass
import concourse.bacc as bacc
import concourse.tile as tile
import concourse.mybir as mybir
from concourse import bass_isa, bass_utils
from concourse.bass_interp import get_hw_module

B, S, H = 2, 4096, 2048
E, TOP_K = 8, 2
I_EXP, I_SH = 1024, 4096
T = B * S
NCORES = 8
I_SLICE = I_SH // NCORES       # 512

P = 128
KT = H // P                    # 16
ITE = I_EXP // P               # 8
ITS = I_SLICE // P             # 4
TB = 512
NBLK = T // TB                 # 16
HH = 512
NHH = H // HH                  # 4
NRANGE = 4
TRANGE = T // NRANGE           # 2048

F32 = mybir.dt.float32
F16 = mybir.dt.float16
I16 = mybir.dt.int16
ALU = mybir.AluOpType
ACTF = mybir.ActivationFunctionType


def build_kernel(pr, group_counts, num_devices=NCORES, with_rs=True,
                 do_expert=True, do_shared=True, debug_out=False):
    """pr: per-range padded counts (multiples of 128, same on all cores).
    group_counts: per-128-group valid-row count (0 => skip scatter)."""
    cap = int(np.ceil(sum(pr) / TB) * TB)          # block-padded capacity
    n_groups = cap // P
    nbe = cap // TB
    # group -> range mapping from pr prefix sums
    g_range = []
    acc = 0
    bounds = np.cumsum(pr)
    for g in range(n_groups):
        lo = g * P
        r = int(np.searchsorted(bounds, lo, side="right"))
        g_range.append(r if r < NRANGE else None)   # None => trailing pad group

    nc = bacc.Bacc(
        "TRN2", target_bir_lowering=False, debug=False, enable_asserts=False,
        num_devices=num_devices, num_swdge_queues=4,
    )
    xT16 = nc.dram_tensor("xT16", [KT, P, T], F16, kind="ExternalInput").ap()
    xTe16 = nc.dram_tensor("xTe16", [KT, P, cap], F16, kind="ExternalInput").ap()
    w9g = nc.dram_tensor("w9g", [KT, P, E], F16, kind="ExternalInput").ap()
    wseg = nc.dram_tensor("wseg", [KT, P, 1], F16, kind="ExternalInput").ap()
    w1e = nc.dram_tensor("w1e", [KT, P, I_EXP], F16, kind="ExternalInput").ap()
    w2e = nc.dram_tensor("w2e", [KT, P, I_EXP], F16, kind="ExternalInput").ap()
    w3e = nc.dram_tensor("w3e", [ITE, P, H], F16, kind="ExternalInput").ap()
    w1s = nc.dram_tensor("w1s", [KT, P, I_SLICE], F16, kind="ExternalInput").ap()
    w2s = nc.dram_tensor("w2s", [KT, P, I_SLICE], F16, kind="ExternalInput").ap()
    w3s = nc.dram_tensor("w3s", [ITS, P, H], F16, kind="ExternalInput").ap()
    sel8d = nc.dram_tensor("sel8", [E, 1], F32, kind="ExternalInput").ap()
    idx16d = nc.dram_tensor("idx16", [P, cap // 16], I16, kind="ExternalInput").ap()
    out_shard = nc.dram_tensor(
        "out_shard", [NRANGE, TRANGE // NCORES, H], F32, kind="ExternalOutput"
    ).ap()

    with tile.TileContext(nc) as tc, ExitStack() as ctx:
        dram = ctx.enter_context(tc.tile_pool(name="dram", bufs=1, space="DRAM"))
        # rows [TRANGE, TRANGE+P) are a garbage region for equalizing pads
        partials = [
            dram.tile([TRANGE + P, H], F32, tag=f"partial{r}", name=f"partial{r}")
            for r in range(NRANGE)
        ]

        # Zeroing 68MB up front would put ~190us of DMA ahead of the first
        # weight/activation loads. Instead emit each range's zero DMAs just
        # before the expert block whose scatters first need that range
        # (glist is range-ordered), so zeroing hides behind compute.
        sbZ = ctx.enter_context(tc.tile_pool(name="sbZ", bufs=1))
        ztile = sbZ.tile([P, H], F32, tag="ztile")
        nc.vector.memset(ztile[:], 0.0)
        zeroed = [False] * NRANGE

        def zero_range(r):
            if r is None or zeroed[r]:
                return
            zeroed[r] = True
            for i in range(TRANGE // P):
                nc.sync.dma_start(partials[r][i * P : (i + 1) * P, :], ztile[:])

        if do_expert:
            _expert_pass(nc, tc, partials, xTe16, w9g, w1e, w2e, w3e, sel8d,
                         idx16d, cap, nbe, g_range, group_counts, zero_range)
        for r in range(NRANGE):
            zero_range(r)
        if do_shared:
            _shared_pass(nc, tc, partials, xT16, wseg, w1s, w2s, w3s)

        # ---- per-range ReduceScatter + output -----------------------------
        with tc.tile_pool(name="dramR", bufs=1, space="DRAM") as dramR:
            if with_rs:
                for r in range(NRANGE):
                    rs_out = dramR.tile(
                        [TRANGE // NCORES, H], F32, tag=f"rsout{r}", name=f"rsout{r}"
                    )
                    nc.gpsimd.collective_compute(
                        "ReduceScatter",
                        ALU.add,
                        replica_groups=[list(range(NCORES))],
                        ins=[partials[r][0:TRANGE, :].opt()],
                        outs=[rs_out.opt()],
                    )
                    nc.sync.dma_start(out_shard[r], rs_out[:])
            elif debug_out:
                dbg = nc.dram_tensor(
                    "partial_dbg", [T, H], F32, kind="ExternalOutput"
                ).ap()
                for r in range(NRANGE):
                    nc.sync.dma_start(
                        dbg[r * TRANGE : (r + 1) * TRANGE, :],
                        partials[r][0:TRANGE, :],
                    )
            else:
                for r in range(NRANGE):
                    nc.sync.dma_start(
                        out_shard[r], partials[r][0 : TRANGE // NCORES, :]
                    )

    nc.compile()
    return nc


def _expert_pass(nc, tc, partials, xTe16, w9g, w1e, w2e, w3e, sel8d, idx16d,
                 cap, nbe, g_range, group_counts, zero_range):
    with (
        tc.tile_pool(name="cstE", bufs=1) as cst,
        tc.tile_pool(name="sbXE", bufs=2) as sbX,
        tc.tile_pool(name="sbHE", bufs=1) as sbH,
        tc.tile_pool(name="sbTE", bufs=2) as sbT,
        tc.tile_pool(name="sbYE", bufs=1) as sbY,
        tc.tile_pool(name="psE", bufs=2, space="PSUM") as ps,
    ):
        w1sb = cst.tile([P, KT, I_EXP], F16, tag="w1sb")
        nc.sync.dma_start(w1sb[:], w1e.rearrange("k p n -> p k n"))
        w2sb = cst.tile([P, KT, I_EXP], F16, tag="w2sb")
        nc.sync.dma_start(w2sb[:], w2e.rearrange("k p n -> p k n"))
        w9sb = cst.tile([P, KT, E], F16, tag="w9sb")
        nc.sync.dma_start(w9sb[:], w9g.rearrange("k p n -> p k n"))
        sel8sb = cst.tile([E, 1], F32, tag="sel8sb")
        nc.sync.dma_start(sel8sb[:], sel8d)
        idxsb = cst.tile([P, cap // 16], I16, tag="idxsb")
        nc.sync.dma_start(idxsb[:], idx16d)
        w3esb = cst.tile([P, ITE, H], F16, tag="w3esb")
        nc.sync.dma_start(w3esb[:], w3e.rearrange("i p h -> p i h"))

        for eb in range(nbe):
            bsl = slice(eb * TB, (eb + 1) * TB)
            xb = sbX.tile([P, KT, TB], F16, tag="xbe")
            nc.sync.dma_start(xb[:], xTe16[:, :, bsl].rearrange("k p t -> p k t"))
            # zero the ranges this block's scatters will touch -- emitted
            # after the activation load so the block's matmuls aren't queued
            # behind 17MB of zero-DMA; the scatters at block end still wait
            # on the zeros via Tile's per-tensor ordering
            for g in range(eb * (TB // P), (eb + 1) * (TB // P)):
                zero_range(g_range[g])

            # recompute this expert's routing weight for gathered tokens:
            # comb = softmax(logits)[e]  (token is in top-2 by construction)
            ps_l = ps.tile([E, TB], F32, tag="ps_l")
            for ko in range(KT):
                nc.tensor.matmul(
                    ps_l[:], w9sb[:, ko, :], xb[:, ko],
                    start=(ko == 0), stop=(ko == KT - 1),
                )
            ex = sbT.tile([E, TB], F32, tag="ex")
            nc.scalar.activation(ex[:], ps_l[:], ACTF.Exp)
            sm = sbT.tile([E, TB], F32, tag="sm")
            nc.gpsimd.partition_all_reduce(
                sm[:], ex[:], channels=E, reduce_op=bass_isa.ReduceOp.add
            )
            rc = sbT.tile([E, TB], F32, tag="rc")
            nc.vector.reciprocal(rc[:], sm[:])
            pr_ = sbT.tile([E, TB], F32, tag="pr")
            nc.vector.tensor_mul(out=pr_[:], in0=ex[:], in1=rc[:])
            nc.vector.tensor_tensor(
                pr_[:], pr_[:], sel8sb[:, 0:1].to_broadcast([E, TB]), ALU.mult
            )
            ce = sbT.tile([E, TB], F32, tag="ce")
            nc.gpsimd.partition_all_reduce(
                ce[:], pr_[:], channels=E, reduce_op=bass_isa.ReduceOp.add
            )
            bc_e = sbT.tile([P, TB], F32, tag="bc_e")
            nc.gpsimd.partition_broadcast(bc_e[:], ce[0:1, :], channels=P)

            hc = sbH.tile([P, ITE, TB], F16, tag="hce")
            for it in range(ITE):
                isl = slice(it * P, (it + 1) * P)
                psG = ps.tile([P, TB], F32, tag="psG")
                psU = ps.tile([P, TB], F32, tag="psU")
                for ko in range(KT):
                    nc.tensor.matmul(
                        psG[:], w1sb[:, ko, isl], xb[:, ko],
                        start=(ko == 0), stop=(ko == KT - 1),
                    )
                for ko in range(KT):
                    nc.tensor.matmul(
                        psU[:], w2sb[:, ko, isl], xb[:, ko],
                        start=(ko == 0), stop=(ko == KT - 1),
                    )
                sg = sbT.tile([P, TB], F32, tag="sg")
                nc.scalar.activation(sg[:], psG[:], ACTF.Silu)
                nc.vector.tensor_mul(out=sg[:], in0=sg[:], in1=psU[:])
                nc.vector.tensor_tensor(hc[:, it], sg[:], bc_e[:], ALU.mult)

            # token-major down-proj: yf[tok, h]
            yf = sbY.tile([P, TB // P, NHH * HH], F32, tag="yf")
            for hh in range(NHH):
                for ts_ in range(TB // P):
                    psY = ps.tile([P, HH], F32, tag="psY")
                    for it in range(ITE):
                        nc.tensor.matmul(
                            psY[:],
                            hc[:, it, ts_ * P : (ts_ + 1) * P],
                            w3esb[:, it, hh * HH : (hh + 1) * HH],
                            start=(it == 0), stop=(it == ITE - 1),
                        )
                    nc.vector.tensor_copy(
                        yf[:, ts_, hh * HH : (hh + 1) * HH], psY[:]
                    )
            # scatter-add 128-row groups into their range's partial
            for ts_ in range(TB // P):
                g = eb * (TB // P) + ts_
                r = g_range[g]
                nvalid = group_counts[g]
                if r is None or nvalid == 0:
                    continue
                nc.gpsimd.dma_scatter_add(
                    out_ap=partials[r][:, :],
                    in_ap=yf[:, ts_ : ts_ + 1, :],
                    idxs_ap=idxsb[:, g * 8 : (g + 1) * 8],
                    num_idxs=P,
                    num_idxs_reg=int(nvalid),
                    elem_size=H,
                    queue_num=g % 4,
                )


def _shared_pass(nc, tc, partials, xT16, wseg, w1s, w2s, w3s):
    with (
        tc.tile_pool(name="cstS", bufs=1) as cst,
        tc.tile_pool(name="sbXS", bufs=2) as sbX,
        tc.tile_pool(name="sbHS", bufs=2) as sbH,
        tc.tile_pool(name="sbTS", bufs=3) as sbT,
        tc.tile_pool(name="sbYS", bufs=2) as sbY,
        tc.tile_pool(name="psS", bufs=2, space="PSUM") as ps,
    ):
        w1sb = cst.tile([P, KT, I_SLICE], F16, tag="w1sbs")
        nc.sync.dma_start(w1sb[:], w1s.rearrange("k p n -> p k n"))
        w2sb = cst.tile([P, KT, I_SLICE], F16, tag="w2sbs")
        nc.sync.dma_start(w2sb[:], w2s.rearrange("k p n -> p k n"))
        wgsb = cst.tile([P, KT, 1], F16, tag="wgsb")
        nc.sync.dma_start(wgsb[:], wseg.rearrange("k p n -> p k n"))
        # shared-expert down weights are small enough to keep resident
        w3sb = cst.tile([P, ITS, H], F16, tag="w3sb")
        nc.sync.dma_start(w3sb[:], w3s.rearrange("i p h -> p i h"))

        for b in range(NBLK):
            bsl = slice(b * TB, (b + 1) * TB)
            rng_i = (b * TB) // TRANGE
            xb = sbX.tile([P, KT, TB], F16, tag="xbs")
            nc.sync.dma_start(xb[:], xT16[:, :, bsl].rearrange("k p t -> p k t"))

            # sigmoid-gate dot product on DVE (PE is the bottleneck engine;
            # 256 M=1 matmuls cost 55us of PE across the pass, DVE is idle)
            acc = sbT.tile([P, TB], F32, tag="gacc")
            tmp = sbT.tile([P, TB], F32, tag="gtmp")
            for ko in range(KT):
                dst = acc if ko == 0 else tmp
                nc.vector.tensor_tensor(
                    dst[:], xb[:, ko],
                    wgsb[:, ko, 0:1].to_broadcast([P, TB]), ALU.mult,
                )
                if ko > 0:
                    nc.vector.tensor_add(out=acc[:], in0=acc[:], in1=tmp[:])
            gsum = sbT.tile([P, TB], F32, tag="gsum")
            nc.gpsimd.partition_all_reduce(
                gsum[:], acc[:], channels=P, reduce_op=bass_isa.ReduceOp.add
            )
            bc_s = sbT.tile([P, TB], F32, tag="bc_s")
            nc.scalar.activation(bc_s[:], gsum[:], ACTF.Sigmoid)

            hc = sbH.tile([P, ITS, TB], F16, tag="hcs")
            for it in range(ITS):
                isl = slice(it * P, (it + 1) * P)
                psG = ps.tile([P, TB], F32, tag="psGs")
                psU = ps.tile([P, TB], F32, tag="psUs")
                for ko in range(KT):
                    nc.tensor.matmul(
                        psG[:], w1sb[:, ko, isl], xb[:, ko],
                        start=(ko == 0), stop=(ko == KT - 1),
                    )
                for ko in range(KT):
                    nc.tensor.matmul(
                        psU[:], w2sb[:, ko, isl], xb[:, ko],
                        start=(ko == 0), stop=(ko == KT - 1),
                    )
                sg = sbT.tile([P, TB], F32, tag="sgs")
                nc.scalar.activation(sg[:], psG[:], ACTF.Silu)
                nc.vector.tensor_mul(out=sg[:], in0=sg[:], in1=psU[:])
                nc.vector.tensor_tensor(hc[:, it], sg[:], bc_s[:], ALU.mult)

            # stage the whole block's [TB, H] in SBUF, then one accum-DMA
            # (per-tile accum-DMAs each hold the GPSIMD engine ~1us for
            # SWDGE descriptor generation -- 256 of them serialize)
            yb = sbY.tile([P, TB // P, H], F32, tag="yblk")
            for hh in range(NHH):
                for ts_ in range(TB // P):
                    psY = ps.tile([P, HH], F32, tag="psYs")
                    for it in range(ITS):
                        nc.tensor.matmul(
                            psY[:],
                            hc[:, it, ts_ * P : (ts_ + 1) * P],
                            w3sb[:, it, hh * HH : (hh + 1) * HH],
                            start=(it == 0), stop=(it == ITS - 1),
                        )
                    nc.vector.tensor_copy(
                        yb[:, ts_, hh * HH : (hh + 1) * HH], psY[:]
                    )
            row0 = (b * TB) % TRANGE
            nc.gpsimd.dma_start(
                partials[rng_i][row0 : row0 + TB, :].rearrange(
                    "(a p) h -> p a h", p=P
                ),
                yb[:],
                accum_op=ALU.add,
            )


def route_host(inputs):
    """Routing decision only (indices); all values are recomputed on device."""
    x = np.asarray(inputs["hidden_states"], np.float64).reshape(T, H)
    gw = np.asarray(inputs["gate_w"], np.float64)
    logits = x @ gw.T
    p = np.exp(logits - logits.max(-1, keepdims=True))
    p /= p.sum(-1, keepdims=True)
    order = np.argsort(-p, axis=-1, kind="stable")
    top2 = order[:, :TOP_K]
    per_core = []
    for e in range(NCORES):
        toks = np.where((top2 == e).any(-1))[0]
        per_range = [
            toks[(toks >= r * TRANGE) & (toks < (r + 1) * TRANGE)] - r * TRANGE
            for r in range(NRANGE)
        ]
        per_core.append(per_range)
    # per-range padded counts: max over cores, rounded up to 128
    pr = [
        int(np.ceil(max(len(per_core[c][r]) for c in range(NCORES)) / P) * P)
        for r in range(NRANGE)
    ]
    return per_core, pr


def _derive_params(inputs):
    """Per-group scatter counts must be identical across cores (one SPMD
    program).  Each group's count = max over cores; cores with fewer tokens
    pad the window with index TRANGE (the garbage row region), so every core
    has exactly `count` non-negative entries followed by -1s."""
    per_core, pr = route_host(inputs)
    cap = int(np.ceil(sum(pr) / TB) * TB)
    n_groups = cap // P
    bounds = np.cumsum(pr)
    group_counts = []
    for g in range(n_groups):
        lo = g * P
        r = int(np.searchsorted(bounds, lo, side="right"))
        if r >= NRANGE:
            group_counts.append(0)
            continue
        seg_lo = lo - (int(bounds[r - 1]) if r > 0 else 0)
        mx = max(
            min(max(len(per_core[c][r]) - seg_lo, 0), P) for c in range(NCORES)
        )
        group_counts.append(int(mx))
    return per_core, pr, cap, group_counts


def build_idx_and_gather(per_range_tok, pr, cap, group_counts):
    """Per-core: padded index list (range-relative; surplus entries within a
    group's count point at the garbage row TRANGE; -1 beyond) + absolute
    gather columns (pads gather token 0)."""
    glist = np.full((cap,), -1, np.int16)
    gcols = np.zeros((cap,), np.int64)
    bounds = np.cumsum(pr)
    off = 0
    for r in range(NRANGE):
        toks = per_range_tok[r]
        glist[off : off + len(toks)] = toks.astype(np.int16)
        gcols[off : off + len(toks)] = toks + r * TRANGE
        off += pr[r]
    # within each group, raise entries [own_count, group_count) to garbage row
    for g in range(cap // P):
        cnt = group_counts[g]
        w = glist[g * P : (g + 1) * P]
        pad = (w < 0) & (np.arange(P) < cnt)
        w[pad] = TRANGE
    idx16 = np.empty((P, cap // 16), np.int16)
    wrapped = glist.reshape(cap // 16, 16).T        # [16, cap/16]
    for k in range(8):
        idx16[k * 16 : (k + 1) * 16] = wrapped
    return glist, gcols, idx16


def make_in_maps(inputs):
    hs = np.ascontiguousarray(inputs["hidden_states"], dtype=np.float32)
    x = hs.reshape(T, H)
    xT16_flat = np.ascontiguousarray(x.T).astype(np.float16)
    xT16 = xT16_flat.reshape(KT, P, T)

    gate_w = np.asarray(inputs["gate_w"], np.float32)
    seg_w = np.asarray(inputs["shared_expert_gate_w"], np.float32)
    w9g = np.ascontiguousarray(gate_w.T).astype(np.float16).reshape(KT, P, E)
    wseg = np.ascontiguousarray(seg_w.T).astype(np.float16).reshape(KT, P, 1)

    egw = np.asarray(inputs["expert_gate_w"], np.float32)
    euw = np.asarray(inputs["expert_up_w"], np.float32)
    edw = np.asarray(inputs["expert_down_w"], np.float32)
    sgw = np.asarray(inputs["shared_gate_w"], np.float32)
    suw = np.asarray(inputs["shared_up_w"], np.float32)
    sdw = np.asarray(inputs["shared_down_w"], np.float32)

    per_core, pr, cap, group_counts = _derive_params(inputs)

    in_maps = []
    for c in range(NCORES):
        ssl = slice(c * I_SLICE, (c + 1) * I_SLICE)
        glist, gcols, idx16 = build_idx_and_gather(
            per_core[c], pr, cap, group_counts
        )
        xTe16 = np.ascontiguousarray(xT16_flat[:, gcols]).reshape(KT, P, cap)
        sel8 = np.zeros((E, 1), np.float32)
        sel8[c, 0] = 1.0
        in_maps.append(
            {
                "xT16": xT16,
                "xTe16": xTe16,
                "w9g": w9g,
                "wseg": wseg,
                "w1e": np.ascontiguousarray(egw[c].T).astype(np.float16).reshape(KT, P, I_EXP),
                "w2e": np.ascontiguousarray(euw[c].T).astype(np.float16).reshape(KT, P, I_EXP),
                "w3e": np.ascontiguousarray(edw[c].T).astype(np.float16).reshape(ITE, P, H),
                "w1s": np.ascontiguousarray(sgw[ssl].T).astype(np.float16).reshape(KT, P, I_SLICE),
                "w2s": np.ascontiguousarray(suw[ssl].T).astype(np.float16).reshape(KT, P, I_SLICE),
                "w3s": np.ascontiguousarray(sdw[:, ssl].T).astype(np.float16).reshape(ITS, P, H),
                "sel8": sel8,
                "idx16": idx16,
            }
        )
    return in_maps, pr, group_counts


def assemble_output(results):
    out = np.empty((T, H), np.float32)
    rows = TRANGE // NCORES
    for c in range(NCORES):
        sh = results[c]["out_shard"]
        for r in range(NRANGE):
            base = r * TRANGE + c * rows
            out[base : base + rows] = sh[r]
    return out.reshape(B, S, H)


_nc_cache = {}


def kernel(**inputs) -> np.ndarray:
    in_maps, pr, group_counts = make_in_maps(inputs)
    key = (tuple(pr), tuple(group_counts))
    if key not in _nc_cache:
        nc = build_kernel(pr, group_counts)
        nc.m = get_hw_module(nc.m)
        _nc_cache[key] = nc
    nc = _nc_cache[key]
    res = bass_utils.run_bass_kernel_spmd(
        nc, in_maps, core_ids=list(range(NCORES))
    )
    return assemble_output(res.results)



# revision 7
# speedup vs baseline: 1.2091x; 1.2091x over previous
"""v3: sparse expert-parallel MoE on 8 trn2 cores.

Core c computes: its expert's FFN on only the tokens routed to it
(host-gathered, padded to the per-range max over cores, unrounded), with
routing weights recomputed on device from the gathered activations; plus a
1/8 tensor-parallel slice of the shared expert over all tokens, gated by an
on-device sigmoid.

Output combine, token-major, one f16 partial [TRANGE, H] per token range:
  1. shared pass runs FIRST and writes its dense [TB, H] block results
     directly (plain DMA, no zeroing, no read-modify-write)
  2. expert pass dma_scatter_add's its weighted rows by in-range token index
     on top (f16 accumulate; garbage row TRANGE absorbs pad slots)
  3. per-range f16 ReduceScatter overlaps trailing expert compute
Host does: routing decision (indices only), gather, reassembly.

v3 vs v2: f16 partials/collective (half the combine traffic, no 68MB zero
pass), shared-before-expert ordering, and exact (unrounded) per-range
capacity with scatter groups split at range boundaries -- cap drops from
2560 to ~2176 rows, saving ~15% of expert-pass matmul work.
"""

import numpy as np
from contextlib import ExitStack

import concourse.bass as bass
import concourse.bacc as bacc
import concourse.tile as tile
import concourse.mybir as mybir
from concourse import bass_isa, bass_utils
from concourse.bass_interp import get_hw_module

B, S, H = 2, 4096, 2048
E, TOP_K = 8, 2
I_EXP, I_SH = 1024, 4096
T = B * S
NCORES = 8
I_SLICE = I_SH // NCORES       # 512

P = 128
KT = H // P                    # 16
ITE = I_EXP // P               # 8
ITS = I_SLICE // P             # 4
TB = 512
NBLK = T // TB                 # 16
HH = 512
NHH = H // HH                  # 4
NRANGE = 4
TRANGE = T // NRANGE           # 2048

F32 = mybir.dt.float32
F16 = mybir.dt.float16
I16 = mybir.dt.int16
ALU = mybir.AluOpType
ACTF = mybir.ActivationFunctionType


def make_calls(pr):
    """Scatter-call plan from exact per-range padded counts.

    Positions [off[r], off[r]+pr[r]) of each core's gathered-token axis hold
    range r's tokens (shorter cores pad with the garbage row). 128-row
    scatter groups that straddle a range boundary are split into one call
    per range; leading rows of a later-range call point at that range's
    garbage row so the in-SBUF source window can stay partition-aligned.
    Returns (cap, calls) with calls = [(group, range, b)] where rows [0, b)
    of the group's window are written (b is the same on every core)."""
    off = np.concatenate([[0], np.cumsum(pr)])
    total = int(off[-1])
    cap = int(np.ceil(total / P) * P)
    calls = []
    for g in range(cap // P):
        lo, hi = g * P, (g + 1) * P
        for r in range(NRANGE):
            s, e = max(lo, int(off[r])), min(hi, int(off[r]) + int(pr[r]))
            if s < e:
                calls.append((g, r, e - lo))
    return cap, calls


def build_kernel(pr, num_devices=NCORES, with_rs=True, debug_out=False):
    """pr: per-range exact padded counts (max over cores, same on all)."""
    cap, calls = make_calls(pr)
    n_groups = cap // P
    nbe = (cap + TB - 1) // TB
    ncols = len(calls) * 8

    nc = bacc.Bacc(
        "TRN2", target_bir_lowering=False, debug=False, enable_asserts=False,
        num_devices=num_devices, num_swdge_queues=4,
    )
    xT16 = nc.dram_tensor("xT16", [KT, P, T], F16, kind="ExternalInput").ap()
    xTe16 = nc.dram_tensor("xTe16", [KT, P, cap], F16, kind="ExternalInput").ap()
    w9g = nc.dram_tensor("w9g", [KT, P, E], F16, kind="ExternalInput").ap()
    wseg = nc.dram_tensor("wseg", [KT, P, 1], F16, kind="ExternalInput").ap()
    w1e = nc.dram_tensor("w1e", [KT, P, I_EXP], F16, kind="ExternalInput").ap()
    w2e = nc.dram_tensor("w2e", [KT, P, I_EXP], F16, kind="ExternalInput").ap()
    w3e = nc.dram_tensor("w3e", [ITE, P, H], F16, kind="ExternalInput").ap()
    w1s = nc.dram_tensor("w1s", [KT, P, I_SLICE], F16, kind="ExternalInput").ap()
    w2s = nc.dram_tensor("w2s", [KT, P, I_SLICE], F16, kind="ExternalInput").ap()
    w3s = nc.dram_tensor("w3s", [ITS, P, H], F16, kind="ExternalInput").ap()
    sel8d = nc.dram_tensor("sel8", [E, 1], F32, kind="ExternalInput").ap()
    idx16d = nc.dram_tensor("idx16", [P, ncols], I16, kind="ExternalInput").ap()
    out_shard = nc.dram_tensor(
        "out_shard", [NRANGE, TRANGE // NCORES, H], F16, kind="ExternalOutput"
    ).ap()

    with tile.TileContext(nc) as tc, ExitStack() as ctx:
        dram = ctx.enter_context(tc.tile_pool(name="dram", bufs=1, space="DRAM"))
        # rows [TRANGE, TRANGE+P) are a garbage region absorbing pad slots
        partials = [
            dram.tile([TRANGE + P, H], F16, tag=f"partial{r}", name=f"partial{r}")
            for r in range(NRANGE)
        ]

        # expert-pass consts that should land before the pass starts; the
        # DMAs are emitted inside _shared_pass after its own critical loads
        cstP = ctx.enter_context(tc.tile_pool(name="cstP", bufs=1))
        w9sb = cstP.tile([P, KT, E], F16, tag="w9sb")
        sel8sb = cstP.tile([E, 1], F32, tag="sel8sb")
        idxsb = cstP.tile([P, ncols], I16, tag="idxsb")
        w1esb = cstP.tile([P, KT, I_EXP], F16, tag="w1sb")
        preload = [
            (w9sb, w9g.rearrange("k p n -> p k n")),
            (sel8sb, sel8d),
            (idxsb, idx16d),
            (w1esb, w1e.rearrange("k p n -> p k n")),
        ]

        _shared_pass(nc, tc, partials, xT16, wseg, w1s, w2s, w3s, preload)
        _expert_pass(nc, tc, partials, xTe16, w2e, w3e,
                     cap, nbe, calls, w9sb, sel8sb, idxsb, w1esb)

        # ---- per-range ReduceScatter + output -----------------------------
        with tc.tile_pool(name="dramR", bufs=1, space="DRAM") as dramR:
            if with_rs:
                for r in range(NRANGE):
                    rs_out = dramR.tile(
                        [TRANGE // NCORES, H], F16, tag=f"rsout{r}", name=f"rsout{r}"
                    )
                    nc.gpsimd.collective_compute(
                        "ReduceScatter",
                        ALU.add,
                        replica_groups=[list(range(NCORES))],
                        ins=[partials[r][0:TRANGE, :].opt()],
                        outs=[rs_out.opt()],
                    )
                    nc.sync.dma_start(out_shard[r], rs_out[:])
            elif debug_out:
                dbg = nc.dram_tensor(
                    "partial_dbg", [T, H], F16, kind="ExternalOutput"
                ).ap()
                for r in range(NRANGE):
                    nc.sync.dma_start(
                        dbg[r * TRANGE : (r + 1) * TRANGE, :],
                        partials[r][0:TRANGE, :],
                    )
            else:
                for r in range(NRANGE):
                    nc.sync.dma_start(
                        out_shard[r], partials[r][0 : TRANGE // NCORES, :]
                    )

    nc.compile()
    return nc


def _shared_pass(nc, tc, partials, xT16, wseg, w1s, w2s, w3s):
    with (
        tc.tile_pool(name="cstS", bufs=1) as cst,
        tc.tile_pool(name="sbXS", bufs=2) as sbX,
        tc.tile_pool(name="sbHS", bufs=2) as sbH,
        tc.tile_pool(name="sbTS", bufs=3) as sbT,
        tc.tile_pool(name="sbYS", bufs=2) as sbY,
        tc.tile_pool(name="psS", bufs=2, space="PSUM") as ps,
    ):
        # load order: tiny gate vec, block-0 activations + gate/up weights
        # (needed immediately), THEN the down weights (needed ~35us in) --
        # the cost model serializes DMA in issue order, so anything queued
        # ahead of the first matmul's operands delays the whole pass
        wgsb = cst.tile([P, KT, 1], F16, tag="wgsb")
        nc.sync.dma_start(wgsb[:], wseg.rearrange("k p n -> p k n"))
        w1sb = cst.tile([P, KT, I_SLICE], F16, tag="w1sbs")
        nc.sync.dma_start(w1sb[:], w1s.rearrange("k p n -> p k n"))
        xb0 = sbX.tile([P, KT, TB], F16, tag="xbs")
        nc.sync.dma_start(xb0[:], xT16[:, :, 0:TB].rearrange("k p t -> p k t"))
        w2sb = cst.tile([P, KT, I_SLICE], F16, tag="w2sbs")
        nc.sync.dma_start(w2sb[:], w2s.rearrange("k p n -> p k n"))
        w3sb = cst.tile([P, ITS, H], F16, tag="w3sb")
        nc.sync.dma_start(w3sb[:], w3s.rearrange("i p h -> p i h"))

        for b in range(NBLK):
            bsl = slice(b * TB, (b + 1) * TB)
            rng_i = (b * TB) // TRANGE
            if b == 0:
                xb = xb0
            else:
                xb = sbX.tile([P, KT, TB], F16, tag="xbs")
                nc.sync.dma_start(
                    xb[:], xT16[:, :, bsl].rearrange("k p t -> p k t")
                )

            # sigmoid-gate dot product on DVE (PE is the bottleneck engine)
            acc = sbT.tile([P, TB], F32, tag="gacc")
            tmp = sbT.tile([P, TB], F32, tag="gtmp")
            for ko in range(KT):
                dst = acc if ko == 0 else tmp
                nc.vector.tensor_tensor(
                    dst[:], xb[:, ko],
                    wgsb[:, ko, 0:1].to_broadcast([P, TB]), ALU.mult,
                )
                if ko > 0:
                    nc.vector.tensor_add(out=acc[:], in0=acc[:], in1=tmp[:])
            gsum = sbT.tile([P, TB], F32, tag="gsum")
            nc.gpsimd.partition_all_reduce(
                gsum[:], acc[:], channels=P, reduce_op=bass_isa.ReduceOp.add
            )
            bc_s = sbT.tile([P, TB], F32, tag="bc_s")
            nc.scalar.activation(bc_s[:], gsum[:], ACTF.Sigmoid)

            hc = sbH.tile([P, ITS, TB], F16, tag="hcs")
            for it in range(ITS):
                isl = slice(it * P, (it + 1) * P)
                psG = ps.tile([P, TB], F32, tag="psGs")
                psU = ps.tile([P, TB], F32, tag="psUs")
                for ko in range(KT):
                    nc.tensor.matmul(
                        psG[:], w1sb[:, ko, isl], xb[:, ko],
                        start=(ko == 0), stop=(ko == KT - 1),
                    )
                for ko in range(KT):
                    nc.tensor.matmul(
                        psU[:], w2sb[:, ko, isl], xb[:, ko],
                        start=(ko == 0), stop=(ko == KT - 1),
                    )
                sg = sbT.tile([P, TB], F32, tag="sgs")
                nc.scalar.activation(sg[:], psG[:], ACTF.Silu)
                nc.vector.tensor_mul(out=sg[:], in0=sg[:], in1=psU[:])
                nc.vector.tensor_tensor(hc[:, it], sg[:], bc_s[:], ALU.mult)

            # stage the block's [TB, H] in SBUF f16, then one plain write DMA
            yb = sbY.tile([P, TB // P, H], F16, tag="yblk")
            for hh in range(NHH):
                for ts_ in range(TB // P):
                    psY = ps.tile([P, HH], F32, tag="psYs")
                    for it in range(ITS):
                        nc.tensor.matmul(
                            psY[:],
                            hc[:, it, ts_ * P : (ts_ + 1) * P],
                            w3sb[:, it, hh * HH : (hh + 1) * HH],
                            start=(it == 0), stop=(it == ITS - 1),
                        )
                    nc.vector.tensor_copy(
                        yb[:, ts_, hh * HH : (hh + 1) * HH], psY[:]
                    )
            row0 = (b * TB) % TRANGE
            nc.sync.dma_start(
                partials[rng_i][row0 : row0 + TB, :].rearrange(
                    "(a p) h -> p a h", p=P
                ),
                yb[:],
            )


def _expert_pass(nc, tc, partials, xTe16, w9g, w1e, w2e, w3e, sel8d, idx16d,
                 cap, nbe, calls, ncols):
    # group -> list of calls for quick lookup. A split (range-boundary)
    # group issues two scatter calls; the ucode may clobber source
    # partitions past a call's count, so the second call reads a duplicate
    # copy of the group's rows staged in an extra yf slot.
    calls_by_block = {}
    extra_per_block = {}
    for ci, (g, r, bcnt) in enumerate(calls):
        eb = g * P // TB
        lst = calls_by_block.setdefault(eb, [])
        ts_ = (g * P - eb * TB) // P
        if any(e[1] == g for e in lst):
            ex = extra_per_block.setdefault(eb, [])
            slot = TB // P + len(ex)
            ex.append((ts_, slot))
        else:
            slot = ts_
        lst.append((ci, g, r, bcnt, slot))
    max_extra = max((len(v) for v in extra_per_block.values()), default=0)

    with (
        tc.tile_pool(name="cstE", bufs=1) as cst,
        tc.tile_pool(name="sbXE", bufs=2) as sbX,
        tc.tile_pool(name="sbHE", bufs=1) as sbH,
        tc.tile_pool(name="sbTE", bufs=2) as sbT,
        tc.tile_pool(name="sbYE", bufs=1) as sbY,
        tc.tile_pool(name="psE", bufs=2, space="PSUM") as ps,
    ):
        # small consts + gate weights first so block 0 matmuls start early
        w9sb = cst.tile([P, KT, E], F16, tag="w9sb")
        nc.sync.dma_start(w9sb[:], w9g.rearrange("k p n -> p k n"))
        sel8sb = cst.tile([E, 1], F32, tag="sel8sb")
        nc.sync.dma_start(sel8sb[:], sel8d)
        idxsb = cst.tile([P, ncols], I16, tag="idxsb")
        nc.sync.dma_start(idxsb[:], idx16d)
        w1sb = cst.tile([P, KT, I_EXP], F16, tag="w1sb")
        nc.sync.dma_start(w1sb[:], w1e.rearrange("k p n -> p k n"))
        w2sb = cst.tile([P, KT, I_EXP], F16, tag="w2sb")
        nc.sync.dma_start(w2sb[:], w2e.rearrange("k p n -> p k n"))
        w3esb = cst.tile([P, ITE, H], F16, tag="w3esb")
        nc.sync.dma_start(w3esb[:], w3e.rearrange("i p h -> p i h"))

        for eb in range(nbe):
            tbb = min(TB, cap - eb * TB)
            bsl = slice(eb * TB, eb * TB + tbb)
            xb = sbX.tile([P, KT, TB], F16, tag="xbe")
            nc.sync.dma_start(
                xb[:, :, 0:tbb], xTe16[:, :, bsl].rearrange("k p t -> p k t")
            )

            # recompute this expert's routing weight for gathered tokens:
            # comb = softmax(logits)[e]  (token is in top-2 by construction)
            ps_l = ps.tile([E, TB], F32, tag="ps_l")
            for ko in range(KT):
                nc.tensor.matmul(
                    ps_l[:, 0:tbb], w9sb[:, ko, :], xb[:, ko, 0:tbb],
                    start=(ko == 0), stop=(ko == KT - 1),
                )
            ex = sbT.tile([E, TB], F32, tag="ex")
            nc.scalar.activation(ex[:, 0:tbb], ps_l[:, 0:tbb], ACTF.Exp)
            sm = sbT.tile([E, TB], F32, tag="sm")
            nc.gpsimd.partition_all_reduce(
                sm[:, 0:tbb], ex[:, 0:tbb], channels=E,
                reduce_op=bass_isa.ReduceOp.add,
            )
            rc = sbT.tile([E, TB], F32, tag="rc")
            nc.vector.reciprocal(rc[:, 0:tbb], sm[:, 0:tbb])
            pr_ = sbT.tile([E, TB], F32, tag="pr")
            nc.vector.tensor_mul(out=pr_[:, 0:tbb], in0=ex[:, 0:tbb],
                                 in1=rc[:, 0:tbb])
            nc.vector.tensor_tensor(
                pr_[:, 0:tbb], pr_[:, 0:tbb],
                sel8sb[:, 0:1].to_broadcast([E, tbb]), ALU.mult
            )
            ce = sbT.tile([E, TB], F32, tag="ce")
            nc.gpsimd.partition_all_reduce(
                ce[:, 0:tbb], pr_[:, 0:tbb], channels=E,
                reduce_op=bass_isa.ReduceOp.add,
            )
            bc_e = sbT.tile([P, TB], F32, tag="bc_e")
            nc.gpsimd.partition_broadcast(
                bc_e[:, 0:tbb], ce[0:1, 0:tbb], channels=P
            )

            hc = sbH.tile([P, ITE, TB], F16, tag="hce")
            for it in range(ITE):
                isl = slice(it * P, (it + 1) * P)
                psG = ps.tile([P, TB], F32, tag="psG")
                psU = ps.tile([P, TB], F32, tag="psU")
                for ko in range(KT):
                    nc.tensor.matmul(
                        psG[:, 0:tbb], w1sb[:, ko, isl], xb[:, ko, 0:tbb],
                        start=(ko == 0), stop=(ko == KT - 1),
                    )
                for ko in range(KT):
                    nc.tensor.matmul(
                        psU[:, 0:tbb], w2sb[:, ko, isl], xb[:, ko, 0:tbb],
                        start=(ko == 0), stop=(ko == KT - 1),
                    )
                sg = sbT.tile([P, TB], F32, tag="sg")
                nc.scalar.activation(sg[:, 0:tbb], psG[:, 0:tbb], ACTF.Silu)
                nc.vector.tensor_mul(out=sg[:, 0:tbb], in0=sg[:, 0:tbb],
                                     in1=psU[:, 0:tbb])
                nc.vector.tensor_tensor(hc[:, it, 0:tbb], sg[:, 0:tbb],
                                        bc_e[:, 0:tbb], ALU.mult)

            # token-major down-proj: yf[tok, h], f16 for the f16 scatter
            yf = sbY.tile([P, TB // P + max_extra, NHH * HH], F16, tag="yf")
            for hh in range(NHH):
                for ts_ in range(tbb // P):
                    psY = ps.tile([P, HH], F32, tag="psY")
                    for it in range(ITE):
                        nc.tensor.matmul(
                            psY[:],
                            hc[:, it, ts_ * P : (ts_ + 1) * P],
                            w3esb[:, it, hh * HH : (hh + 1) * HH],
                            start=(it == 0), stop=(it == ITE - 1),
                        )
                    nc.vector.tensor_copy(
                        yf[:, ts_, hh * HH : (hh + 1) * HH], psY[:]
                    )
            # duplicate boundary-group rows into their extra slots
            for src_ts, slot in extra_per_block.get(eb, []):
                nc.vector.tensor_copy(yf[:, slot, :], yf[:, src_ts, :])
            # scatter-add each group's rows into its range's partial;
            # boundary groups issue one call per overlapped range
            for ci, g, r, bcnt, slot in calls_by_block.get(eb, []):
                nc.gpsimd.dma_scatter_add(
                    out_ap=partials[r][:, :],
                    in_ap=yf[:, slot : slot + 1, :],
                    idxs_ap=idxsb[:, ci * 8 : (ci + 1) * 8],
                    num_idxs=P,
                    num_idxs_reg=int(bcnt),
                    elem_size=H,
                    queue_num=ci % 4,
                )


def route_host(inputs):
    """Routing decision only (indices); all values are recomputed on device."""
    x = np.asarray(inputs["hidden_states"], np.float64).reshape(T, H)
    gw = np.asarray(inputs["gate_w"], np.float64)
    logits = x @ gw.T
    p = np.exp(logits - logits.max(-1, keepdims=True))
    p /= p.sum(-1, keepdims=True)
    order = np.argsort(-p, axis=-1, kind="stable")
    top2 = order[:, :TOP_K]
    per_core = []
    for e in range(NCORES):
        toks = np.where((top2 == e).any(-1))[0]
        per_range = [
            toks[(toks >= r * TRANGE) & (toks < (r + 1) * TRANGE)] - r * TRANGE
            for r in range(NRANGE)
        ]
        per_core.append(per_range)
    # per-range padded counts: exact max over cores (no rounding)
    pr = [
        max(len(per_core[c][r]) for c in range(NCORES))
        for r in range(NRANGE)
    ]
    return per_core, pr


def build_idx_and_gather(per_range_tok, pr, cap, calls):
    """Per-core: position->row values (range-relative; pad slots use the
    garbage row TRANGE), per-call wrapped index columns, and absolute gather
    columns (pads gather token 0)."""
    off = np.concatenate([[0], np.cumsum(pr)]).astype(int)
    pos_val = np.full(cap, -1, np.int64)
    gcols = np.zeros(cap, np.int64)
    for r in range(NRANGE):
        toks = per_range_tok[r]
        base = off[r]
        pos_val[base : base + pr[r]] = TRANGE          # garbage default
        pos_val[base : base + len(toks)] = toks
        gcols[base : base + len(toks)] = toks + r * TRANGE
    idx16 = np.zeros((P, len(calls) * 8), np.int16)
    for ci, (g, r, bcnt) in enumerate(calls):
        lo = g * P
        s = max(lo, int(off[r]))
        a = s - lo
        w = np.full(P, -1, np.int16)
        w[0:a] = TRANGE                                # filler -> garbage
        w[a:bcnt] = pos_val[lo + a : lo + bcnt].astype(np.int16)
        mat = w.reshape(8, 16).T
        for k in range(8):
            idx16[k * 16 : (k + 1) * 16, ci * 8 : (ci + 1) * 8] = mat
    return gcols, idx16


def make_in_maps(inputs):
    hs = np.ascontiguousarray(inputs["hidden_states"], dtype=np.float32)
    x = hs.reshape(T, H)
    xT16_flat = np.ascontiguousarray(x.T).astype(np.float16)
    xT16 = xT16_flat.reshape(KT, P, T)

    gate_w = np.asarray(inputs["gate_w"], np.float32)
    seg_w = np.asarray(inputs["shared_expert_gate_w"], np.float32)
    w9g = np.ascontiguousarray(gate_w.T).astype(np.float16).reshape(KT, P, E)
    wseg = np.ascontiguousarray(seg_w.T).astype(np.float16).reshape(KT, P, 1)

    egw = np.asarray(inputs["expert_gate_w"], np.float32)
    euw = np.asarray(inputs["expert_up_w"], np.float32)
    edw = np.asarray(inputs["expert_down_w"], np.float32)
    sgw = np.asarray(inputs["shared_gate_w"], np.float32)
    suw = np.asarray(inputs["shared_up_w"], np.float32)
    sdw = np.asarray(inputs["shared_down_w"], np.float32)

    per_core, pr = route_host(inputs)
    cap, calls = make_calls(pr)

    in_maps = []
    for c in range(NCORES):
        ssl = slice(c * I_SLICE, (c + 1) * I_SLICE)
        gcols, idx16 = build_idx_and_gather(per_core[c], pr, cap, calls)
        xTe16 = np.ascontiguousarray(xT16_flat[:, gcols]).reshape(KT, P, cap)
        sel8 = np.zeros((E, 1), np.float32)
        sel8[c, 0] = 1.0
        in_maps.append(
            {
                "xT16": xT16,
                "xTe16": xTe16,
                "w9g": w9g,
                "wseg": wseg,
                "w1e": np.ascontiguousarray(egw[c].T).astype(np.float16).reshape(KT, P, I_EXP),
                "w2e": np.ascontiguousarray(euw[c].T).astype(np.float16).reshape(KT, P, I_EXP),
                "w3e": np.ascontiguousarray(edw[c].T).astype(np.float16).reshape(ITE, P, H),
                "w1s": np.ascontiguousarray(sgw[ssl].T).astype(np.float16).reshape(KT, P, I_SLICE),
                "w2s": np.ascontiguousarray(suw[ssl].T).astype(np.float16).reshape(KT, P, I_SLICE),
                "w3s": np.ascontiguousarray(sdw[:, ssl].T).astype(np.float16).reshape(ITS, P, H),
                "sel8": sel8,
                "idx16": idx16,
            }
        )
    return in_maps, pr


def assemble_output(results):
    out = np.empty((T, H), np.float32)
    rows = TRANGE // NCORES
    for c in range(NCORES):
        sh = results[c]["out_shard"]
        for r in range(NRANGE):
            base = r * TRANGE + c * rows
            out[base : base + rows] = np.asarray(sh[r], np.float32)
    return out.reshape(B, S, H)


_nc_cache = {}


def kernel(**inputs) -> np.ndarray:
    in_maps, pr = make_in_maps(inputs)
    key = tuple(pr)
    if key not in _nc_cache:
        nc = build_kernel(pr)
        nc.m = get_hw_module(nc.m)
        _nc_cache[key] = nc
    nc = _nc_cache[key]
    res = bass_utils.run_bass_kernel_spmd(
        nc, in_maps, core_ids=list(range(NCORES))
    )
    return assemble_output(res.results)


# revision 15
# speedup vs baseline: 1.2555x; 1.0383x over previous
"""v3: sparse expert-parallel MoE on 8 trn2 cores.

Core c computes: its expert's FFN on only the tokens routed to it
(host-gathered, padded to the per-range max over cores, unrounded), with
routing weights recomputed on device from the gathered activations; plus a
1/8 tensor-parallel slice of the shared expert over all tokens, gated by an
on-device sigmoid.

Output combine, token-major, one f16 partial [TRANGE, H] per token range:
  1. shared pass runs FIRST and writes its dense [TB, H] block results
     directly (plain DMA, no zeroing, no read-modify-write)
  2. expert pass dma_scatter_add's its weighted rows by in-range token index
     on top (f16 accumulate; garbage row TRANGE absorbs pad slots)
  3. per-range f16 ReduceScatter overlaps trailing expert compute
Host does: routing decision (indices only), gather, reassembly.

v3 vs v2: f16 partials/collective (half the combine traffic, no 68MB zero
pass), shared-before-expert ordering, and exact (unrounded) per-range
capacity with scatter groups split at range boundaries -- cap drops from
2560 to ~2176 rows, saving ~15% of expert-pass matmul work.
"""

import numpy as np
from contextlib import ExitStack

import concourse.bass as bass
import concourse.bacc as bacc
import concourse.tile as tile
import concourse.mybir as mybir
from concourse import bass_isa, bass_utils
from concourse.bass_interp import get_hw_module

B, S, H = 2, 4096, 2048
E, TOP_K = 8, 2
I_EXP, I_SH = 1024, 4096
T = B * S
NCORES = 8
I_SLICE = I_SH // NCORES       # 512

P = 128
KT = H // P                    # 16
ITE = I_EXP // P               # 8
ITS = I_SLICE // P             # 4
TB = 512
NBLK = T // TB                 # 16
HH = 512
NHH = H // HH                  # 4
NRANGE = 4
TRANGE = T // NRANGE           # 2048

F32 = mybir.dt.float32
F16 = mybir.dt.float16
I16 = mybir.dt.int16
ALU = mybir.AluOpType
ACTF = mybir.ActivationFunctionType


def make_calls(pr):
    """Scatter-call plan from exact per-range padded counts.

    Positions [off[r], off[r]+pr[r]) of each core's gathered-token axis hold
    range r's tokens (shorter cores pad with the garbage row). 128-row
    scatter groups that straddle a range boundary are split into one call
    per range; leading rows of a later-range call point at that range's
    garbage row so the in-SBUF source window can stay partition-aligned.
    Returns (cap, calls) with calls = [(group, range, b)] where rows [0, b)
    of the group's window are written (b is the same on every core)."""
    off = np.concatenate([[0], np.cumsum(pr)])
    total = int(off[-1])
    cap = int(np.ceil(total / P) * P)
    calls = []
    for g in range(cap // P):
        lo, hi = g * P, (g + 1) * P
        for r in range(NRANGE):
            s, e = max(lo, int(off[r])), min(hi, int(off[r]) + int(pr[r]))
            if s < e:
                calls.append((g, r, e - lo))
    return cap, calls


def build_kernel(pr, num_devices=NCORES, with_rs=True, debug_out=False):
    """pr: per-range exact padded counts (max over cores, same on all)."""
    cap, calls = make_calls(pr)
    n_groups = cap // P
    nbe = (cap + TB - 1) // TB
    ncols = len(calls) * 8

    nc = bacc.Bacc(
        "TRN2", target_bir_lowering=False, debug=False, enable_asserts=False,
        num_devices=num_devices, num_swdge_queues=4,
    )
    xT16 = nc.dram_tensor("xT16", [KT, P, T], F16, kind="ExternalInput").ap()
    xTe16 = nc.dram_tensor("xTe16", [KT, P, cap], F16, kind="ExternalInput").ap()
    w9g = nc.dram_tensor("w9g", [KT, P, E], F16, kind="ExternalInput").ap()
    wseg = nc.dram_tensor("wseg", [KT, P, 1], F16, kind="ExternalInput").ap()
    w1e = nc.dram_tensor("w1e", [KT, P, I_EXP], F16, kind="ExternalInput").ap()
    w2e = nc.dram_tensor("w2e", [KT, P, I_EXP], F16, kind="ExternalInput").ap()
    w3e = nc.dram_tensor("w3e", [ITE, P, H], F16, kind="ExternalInput").ap()
    w1s = nc.dram_tensor("w1s", [KT, P, I_SLICE], F16, kind="ExternalInput").ap()
    w2s = nc.dram_tensor("w2s", [KT, P, I_SLICE], F16, kind="ExternalInput").ap()
    w3s = nc.dram_tensor("w3s", [ITS, P, H], F16, kind="ExternalInput").ap()
    sel8d = nc.dram_tensor("sel8", [E, 1], F32, kind="ExternalInput").ap()
    idx16d = nc.dram_tensor("idx16", [P, ncols], I16, kind="ExternalInput").ap()
    out_shard = nc.dram_tensor(
        "out_shard", [NRANGE, TRANGE // NCORES, H], F16, kind="ExternalOutput"
    ).ap()

    with tile.TileContext(nc) as tc, ExitStack() as ctx:
        dram = ctx.enter_context(tc.tile_pool(name="dram", bufs=1, space="DRAM"))
        # rows [TRANGE, TRANGE+P) are a garbage region absorbing pad slots
        partials = [
            dram.tile([TRANGE + P, H], F16, tag=f"partial{r}", name=f"partial{r}")
            for r in range(NRANGE)
        ]

        # expert-pass consts that should land before the pass starts; the
        # DMAs are emitted inside _shared_pass after its own critical loads
        cstP = ctx.enter_context(tc.tile_pool(name="cstP", bufs=1))
        w9sb = cstP.tile([P, KT, E], F16, tag="w9sb")
        sel8sb = cstP.tile([E, 1], F32, tag="sel8sb")
        idxsb = cstP.tile([P, ncols], I16, tag="idxsb")
        w1esb = cstP.tile([P, KT, I_EXP], F16, tag="w1sb")
        xbe0 = cstP.tile([P, KT, TB], F16, tag="xbe0")
        w2esb = cstP.tile([P, KT, I_EXP], F16, tag="w2sb")
        preload = [
            (w9sb, w9g.rearrange("k p n -> p k n")),
            (sel8sb, sel8d),
            (idxsb, idx16d),
            (w1esb, w1e.rearrange("k p n -> p k n")),
            (xbe0, xTe16[:, :, 0:TB].rearrange("k p t -> p k t")),
            (w2esb, w2e.rearrange("k p n -> p k n")),
        ]

        _shared_pass(nc, tc, partials, xT16, wseg, w1s, w2s, w3s, preload)
        _expert_pass(nc, tc, partials, xTe16, w2e, w3e,
                     cap, nbe, calls, w9sb, sel8sb, idxsb, w1esb, xbe0)

        # ---- per-range ReduceScatter + output -----------------------------
        with tc.tile_pool(name="dramR", bufs=1, space="DRAM") as dramR:
            if with_rs:
                for r in range(NRANGE):
                    rs_out = dramR.tile(
                        [TRANGE // NCORES, H], F16, tag=f"rsout{r}", name=f"rsout{r}"
                    )
                    nc.gpsimd.collective_compute(
                        "ReduceScatter",
                        ALU.add,
                        replica_groups=[list(range(NCORES))],
                        ins=[partials[r][0:TRANGE, :].opt()],
                        outs=[rs_out.opt()],
                    )
                    nc.sync.dma_start(out_shard[r], rs_out[:])
            elif debug_out:
                dbg = nc.dram_tensor(
                    "partial_dbg", [T, H], F16, kind="ExternalOutput"
                ).ap()
                for r in range(NRANGE):
                    nc.sync.dma_start(
                        dbg[r * TRANGE : (r + 1) * TRANGE, :],
                        partials[r][0:TRANGE, :],
                    )
            else:
                for r in range(NRANGE):
                    nc.sync.dma_start(
                        out_shard[r], partials[r][0 : TRANGE // NCORES, :]
                    )

    nc.compile()
    return nc


def _shared_pass(nc, tc, partials, xT16, wseg, w1s, w2s, w3s, preload):
    with (
        tc.tile_pool(name="cstS", bufs=1) as cst,
        tc.tile_pool(name="sbXS", bufs=2) as sbX,
        tc.tile_pool(name="sbHS", bufs=2) as sbH,
        tc.tile_pool(name="sbTS", bufs=3) as sbT,
        tc.tile_pool(name="sbYS", bufs=2) as sbY,
        tc.tile_pool(name="psS", bufs=2, space="PSUM") as ps,
    ):
        # load order: tiny gate vec, block-0 activations + gate/up weights
        # (needed immediately), THEN the down weights (needed ~35us in) --
        # the cost model serializes DMA in issue order, so anything queued
        # ahead of the first matmul's operands delays the whole pass
        wgsb = cst.tile([P, KT, 1], F16, tag="wgsb")
        nc.sync.dma_start(wgsb[:], wseg.rearrange("k p n -> p k n"))
        w1sb = cst.tile([P, KT, I_SLICE], F16, tag="w1sbs")
        nc.sync.dma_start(w1sb[:], w1s.rearrange("k p n -> p k n"))
        xb0 = sbX.tile([P, KT, TB], F16, tag="xbs")
        nc.sync.dma_start(xb0[:], xT16[:, :, 0:TB].rearrange("k p t -> p k t"))
        w2sb = cst.tile([P, KT, I_SLICE], F16, tag="w2sbs")
        nc.sync.dma_start(w2sb[:], w2s.rearrange("k p n -> p k n"))
        w3sb = cst.tile([P, ITS, H], F16, tag="w3sb")
        nc.sync.dma_start(w3sb[:], w3s.rearrange("i p h -> p i h"))
        # expert-pass preloads ride the idle DMA window behind block 0/1
        for tile_, src in preload:
            nc.sync.dma_start(tile_[:], src)

        for b in range(NBLK):
            bsl = slice(b * TB, (b + 1) * TB)
            rng_i = (b * TB) // TRANGE
            if b == 0:
                xb = xb0
            else:
                xb = sbX.tile([P, KT, TB], F16, tag="xbs")
                nc.sync.dma_start(
                    xb[:], xT16[:, :, bsl].rearrange("k p t -> p k t")
                )

            # sigmoid-gate dot product on DVE (PE is the bottleneck engine)
            acc = sbT.tile([P, TB], F32, tag="gacc")
            tmp = sbT.tile([P, TB], F32, tag="gtmp")
            for ko in range(KT):
                dst = acc if ko == 0 else tmp
                nc.vector.tensor_tensor(
                    dst[:], xb[:, ko],
                    wgsb[:, ko, 0:1].to_broadcast([P, TB]), ALU.mult,
                )
                if ko > 0:
                    nc.vector.tensor_add(out=acc[:], in0=acc[:], in1=tmp[:])
            gsum = sbT.tile([P, TB], F32, tag="gsum")
            nc.gpsimd.partition_all_reduce(
                gsum[:], acc[:], channels=P, reduce_op=bass_isa.ReduceOp.add
            )
            bc_s = sbT.tile([P, TB], F32, tag="bc_s")
            nc.scalar.activation(bc_s[:], gsum[:], ACTF.Sigmoid)

            hc = sbH.tile([P, ITS, TB], F16, tag="hcs")
            for it in range(ITS):
                isl = slice(it * P, (it + 1) * P)
                psG = ps.tile([P, TB], F32, tag="psGs")
                psU = ps.tile([P, TB], F32, tag="psUs")
                for ko in range(KT):
                    nc.tensor.matmul(
                        psG[:], w1sb[:, ko, isl], xb[:, ko],
                        start=(ko == 0), stop=(ko == KT - 1),
                    )
                for ko in range(KT):
                    nc.tensor.matmul(
                        psU[:], w2sb[:, ko, isl], xb[:, ko],
                        start=(ko == 0), stop=(ko == KT - 1),
                    )
                sg = sbT.tile([P, TB], F32, tag="sgs")
                nc.scalar.activation(sg[:], psG[:], ACTF.Silu)
                nc.vector.tensor_mul(out=sg[:], in0=sg[:], in1=psU[:])
                nc.vector.tensor_tensor(hc[:, it], sg[:], bc_s[:], ALU.mult)

            # stage the block's [TB, H] in SBUF f16, then one plain write DMA
            yb = sbY.tile([P, TB // P, H], F16, tag="yblk")
            for hh in range(NHH):
                for ts_ in range(TB // P):
                    psY = ps.tile([P, HH], F32, tag="psYs")
                    for it in range(ITS):
                        nc.tensor.matmul(
                            psY[:],
                            hc[:, it, ts_ * P : (ts_ + 1) * P],
                            w3sb[:, it, hh * HH : (hh + 1) * HH],
                            start=(it == 0), stop=(it == ITS - 1),
                        )
                    nc.vector.tensor_copy(
                        yb[:, ts_, hh * HH : (hh + 1) * HH], psY[:]
                    )
            row0 = (b * TB) % TRANGE
            nc.sync.dma_start(
                partials[rng_i][row0 : row0 + TB, :].rearrange(
                    "(a p) h -> p a h", p=P
                ),
                yb[:],
            )


def _expert_pass(nc, tc, partials, xTe16, w2e, w3e,
                 cap, nbe, calls, w9sb, sel8sb, idxsb, w1sb, xbe0):
    # group -> list of calls for quick lookup. A split (range-boundary)
    # group issues two scatter calls; the ucode may clobber source
    # partitions past a call's count, so the second call reads a duplicate
    # copy of the group's rows staged in an extra yf slot.
    calls_by_block = {}
    extra_per_block = {}
    for ci, (g, r, bcnt) in enumerate(calls):
        eb = g * P // TB
        lst = calls_by_block.setdefault(eb, [])
        ts_ = (g * P - eb * TB) // P
        if any(e[1] == g for e in lst):
            ex = extra_per_block.setdefault(eb, [])
            slot = TB // P + len(ex)
            ex.append((ts_, slot))
        else:
            slot = ts_
        lst.append((ci, g, r, bcnt, slot))
    max_extra = max((len(v) for v in extra_per_block.values()), default=0)

    with (
        tc.tile_pool(name="cstE", bufs=1) as cst,
        tc.tile_pool(name="sbXE", bufs=2) as sbX,
        tc.tile_pool(name="sbHE", bufs=1) as sbH,
        tc.tile_pool(name="sbTE", bufs=2) as sbT,
        tc.tile_pool(name="sbYE", bufs=1) as sbY,
        tc.tile_pool(name="psE", bufs=2, space="PSUM") as ps,
    ):
        # w9/sel/idx/w1/xbe0 preloaded during the shared pass
        xb0 = xbe0
        w2sb = cst.tile([P, KT, I_EXP], F16, tag="w2sb")
        nc.sync.dma_start(w2sb[:], w2e.rearrange("k p n -> p k n"))
        w3esb = cst.tile([P, ITE, H], F16, tag="w3esb")
        nc.sync.dma_start(w3esb[:], w3e.rearrange("i p h -> p i h"))

        for eb in range(nbe):
            tbb = min(TB, cap - eb * TB)
            bsl = slice(eb * TB, eb * TB + tbb)
            if eb == 0:
                xb = xb0
            else:
                xb = sbX.tile([P, KT, TB], F16, tag="xbe")
                nc.sync.dma_start(
                    xb[:, :, 0:tbb], xTe16[:, :, bsl].rearrange("k p t -> p k t")
                )

            # recompute this expert's routing weight for gathered tokens:
            # comb = softmax(logits)[e]  (token is in top-2 by construction)
            ps_l = ps.tile([E, TB], F32, tag="ps_l")
            for ko in range(KT):
                nc.tensor.matmul(
                    ps_l[:, 0:tbb], w9sb[:, ko, :], xb[:, ko, 0:tbb],
                    start=(ko == 0), stop=(ko == KT - 1),
                )
            ex = sbT.tile([E, TB], F32, tag="ex")
            nc.scalar.activation(ex[:, 0:tbb], ps_l[:, 0:tbb], ACTF.Exp)
            sm = sbT.tile([E, TB], F32, tag="sm")
            nc.gpsimd.partition_all_reduce(
                sm[:, 0:tbb], ex[:, 0:tbb], channels=E,
                reduce_op=bass_isa.ReduceOp.add,
            )
            rc = sbT.tile([E, TB], F32, tag="rc")
            nc.vector.reciprocal(rc[:, 0:tbb], sm[:, 0:tbb])
            pr_ = sbT.tile([E, TB], F32, tag="pr")
            nc.vector.tensor_mul(out=pr_[:, 0:tbb], in0=ex[:, 0:tbb],
                                 in1=rc[:, 0:tbb])
            nc.vector.tensor_tensor(
                pr_[:, 0:tbb], pr_[:, 0:tbb],
                sel8sb[:, 0:1].to_broadcast([E, tbb]), ALU.mult
            )
            ce = sbT.tile([E, TB], F32, tag="ce")
            nc.gpsimd.partition_all_reduce(
                ce[:, 0:tbb], pr_[:, 0:tbb], channels=E,
                reduce_op=bass_isa.ReduceOp.add,
            )
            bc_e = sbT.tile([P, TB], F32, tag="bc_e")
            nc.gpsimd.partition_broadcast(
                bc_e[:, 0:tbb], ce[0:1, 0:tbb], channels=P
            )

            hc = sbH.tile([P, ITE, TB], F16, tag="hce")
            for it in range(ITE):
                isl = slice(it * P, (it + 1) * P)
                psG = ps.tile([P, TB], F32, tag="psG")
                psU = ps.tile([P, TB], F32, tag="psU")
                for ko in range(KT):
                    nc.tensor.matmul(
                        psG[:, 0:tbb], w1sb[:, ko, isl], xb[:, ko, 0:tbb],
                        start=(ko == 0), stop=(ko == KT - 1),
                    )
                for ko in range(KT):
                    nc.tensor.matmul(
                        psU[:, 0:tbb], w2sb[:, ko, isl], xb[:, ko, 0:tbb],
                        start=(ko == 0), stop=(ko == KT - 1),
                    )
                sg = sbT.tile([P, TB], F32, tag="sg")
                nc.scalar.activation(sg[:, 0:tbb], psG[:, 0:tbb], ACTF.Silu)
                nc.vector.tensor_mul(out=sg[:, 0:tbb], in0=sg[:, 0:tbb],
                                     in1=psU[:, 0:tbb])
                nc.vector.tensor_tensor(hc[:, it, 0:tbb], sg[:, 0:tbb],
                                        bc_e[:, 0:tbb], ALU.mult)

            # token-major down-proj: yf[tok, h], f16 for the f16 scatter
            yf = sbY.tile([P, TB // P + max_extra, NHH * HH], F16, tag="yf")
            for hh in range(NHH):
                for ts_ in range(tbb // P):
                    psY = ps.tile([P, HH], F32, tag="psY")
                    for it in range(ITE):
                        nc.tensor.matmul(
                            psY[:],
                            hc[:, it, ts_ * P : (ts_ + 1) * P],
                            w3esb[:, it, hh * HH : (hh + 1) * HH],
                            start=(it == 0), stop=(it == ITE - 1),
                        )
                    nc.vector.tensor_copy(
                        yf[:, ts_, hh * HH : (hh + 1) * HH], psY[:]
                    )
            # duplicate boundary-group rows into their extra slots
            for src_ts, slot in extra_per_block.get(eb, []):
                nc.vector.tensor_copy(yf[:, slot, :], yf[:, src_ts, :])
            # scatter-add each group's rows into its range's partial;
            # boundary groups issue one call per overlapped range
            for ci, g, r, bcnt, slot in calls_by_block.get(eb, []):
                nc.gpsimd.dma_scatter_add(
                    out_ap=partials[r][:, :],
                    in_ap=yf[:, slot : slot + 1, :],
                    idxs_ap=idxsb[:, ci * 8 : (ci + 1) * 8],
                    num_idxs=P,
                    num_idxs_reg=int(bcnt),
                    elem_size=H,
                    queue_num=ci % 4,
                )


def route_host(inputs):
    """Routing decision only (indices); all values are recomputed on device."""
    x = np.asarray(inputs["hidden_states"], np.float64).reshape(T, H)
    gw = np.asarray(inputs["gate_w"], np.float64)
    logits = x @ gw.T
    p = np.exp(logits - logits.max(-1, keepdims=True))
    p /= p.sum(-1, keepdims=True)
    order = np.argsort(-p, axis=-1, kind="stable")
    top2 = order[:, :TOP_K]
    per_core = []
    for e in range(NCORES):
        toks = np.where((top2 == e).any(-1))[0]
        per_range = [
            toks[(toks >= r * TRANGE) & (toks < (r + 1) * TRANGE)] - r * TRANGE
            for r in range(NRANGE)
        ]
        per_core.append(per_range)
    # per-range padded counts: exact max over cores (no rounding)
    pr = [
        max(len(per_core[c][r]) for c in range(NCORES))
        for r in range(NRANGE)
    ]
    return per_core, pr


def build_idx_and_gather(per_range_tok, pr, cap, calls):
    """Per-core: position->row values (range-relative; pad slots use the
    garbage row TRANGE), per-call wrapped index columns, and absolute gather
    columns (pads gather token 0)."""
    off = np.concatenate([[0], np.cumsum(pr)]).astype(int)
    pos_val = np.full(cap, -1, np.int64)
    gcols = np.zeros(cap, np.int64)
    for r in range(NRANGE):
        toks = per_range_tok[r]
        base = off[r]
        pos_val[base : base + pr[r]] = TRANGE          # garbage default
        pos_val[base : base + len(toks)] = toks
        gcols[base : base + len(toks)] = toks + r * TRANGE
    idx16 = np.zeros((P, len(calls) * 8), np.int16)
    for ci, (g, r, bcnt) in enumerate(calls):
        lo = g * P
        s = max(lo, int(off[r]))
        a = s - lo
        w = np.full(P, -1, np.int16)
        w[0:a] = TRANGE                                # filler -> garbage
        w[a:bcnt] = pos_val[lo + a : lo + bcnt].astype(np.int16)
        mat = w.reshape(8, 16).T
        for k in range(8):
            idx16[k * 16 : (k + 1) * 16, ci * 8 : (ci + 1) * 8] = mat
    return gcols, idx16


def make_in_maps(inputs):
    hs = np.ascontiguousarray(inputs["hidden_states"], dtype=np.float32)
    x = hs.reshape(T, H)
    xT16_flat = np.ascontiguousarray(x.T).astype(np.float16)
    xT16 = xT16_flat.reshape(KT, P, T)

    gate_w = np.asarray(inputs["gate_w"], np.float32)
    seg_w = np.asarray(inputs["shared_expert_gate_w"], np.float32)
    w9g = np.ascontiguousarray(gate_w.T).astype(np.float16).reshape(KT, P, E)
    wseg = np.ascontiguousarray(seg_w.T).astype(np.float16).reshape(KT, P, 1)

    egw = np.asarray(inputs["expert_gate_w"], np.float32)
    euw = np.asarray(inputs["expert_up_w"], np.float32)
    edw = np.asarray(inputs["expert_down_w"], np.float32)
    sgw = np.asarray(inputs["shared_gate_w"], np.float32)
    suw = np.asarray(inputs["shared_up_w"], np.float32)
    sdw = np.asarray(inputs["shared_down_w"], np.float32)

    per_core, pr = route_host(inputs)
    cap, calls = make_calls(pr)

    in_maps = []
    for c in range(NCORES):
        ssl = slice(c * I_SLICE, (c + 1) * I_SLICE)
        gcols, idx16 = build_idx_and_gather(per_core[c], pr, cap, calls)
        xTe16 = np.ascontiguousarray(xT16_flat[:, gcols]).reshape(KT, P, cap)
        sel8 = np.zeros((E, 1), np.float32)
        sel8[c, 0] = 1.0
        in_maps.append(
            {
                "xT16": xT16,
                "xTe16": xTe16,
                "w9g": w9g,
                "wseg": wseg,
                "w1e": np.ascontiguousarray(egw[c].T).astype(np.float16).reshape(KT, P, I_EXP),
                "w2e": np.ascontiguousarray(euw[c].T).astype(np.float16).reshape(KT, P, I_EXP),
                "w3e": np.ascontiguousarray(edw[c].T).astype(np.float16).reshape(ITE, P, H),
                "w1s": np.ascontiguousarray(sgw[ssl].T).astype(np.float16).reshape(KT, P, I_SLICE),
                "w2s": np.ascontiguousarray(suw[ssl].T).astype(np.float16).reshape(KT, P, I_SLICE),
                "w3s": np.ascontiguousarray(sdw[:, ssl].T).astype(np.float16).reshape(ITS, P, H),
                "sel8": sel8,
                "idx16": idx16,
            }
        )
    return in_maps, pr


def assemble_output(results):
    out = np.empty((T, H), np.float32)
    rows = TRANGE // NCORES
    for c in range(NCORES):
        sh = results[c]["out_shard"]
        for r in range(NRANGE):
            base = r * TRANGE + c * rows
            out[base : base + rows] = np.asarray(sh[r], np.float32)
    return out.reshape(B, S, H)


_nc_cache = {}


def kernel(**inputs) -> np.ndarray:
    in_maps, pr = make_in_maps(inputs)
    key = tuple(pr)
    if key not in _nc_cache:
        nc = build_kernel(pr)
        nc.m = get_hw_module(nc.m)
        _nc_cache[key] = nc
    nc = _nc_cache[key]
    res = bass_utils.run_bass_kernel_spmd(
        nc, in_maps, core_ids=list(range(NCORES))
    )
    return assemble_output(res.results)


# revision 22
# speedup vs baseline: 1.3003x; 1.0357x over previous
"""v3: sparse expert-parallel MoE on 8 trn2 cores.

Core c computes: its expert's FFN on only the tokens routed to it
(host-gathered, padded to the per-range max over cores, unrounded), with
routing weights recomputed on device from the gathered activations; plus a
1/8 tensor-parallel slice of the shared expert over all tokens, gated by an
on-device sigmoid.

Output combine, token-major, one f16 partial [TRANGE, H] per token range:
  1. shared pass runs FIRST and writes its dense [TB, H] block results
     directly (plain DMA, no zeroing, no read-modify-write)
  2. expert pass dma_scatter_add's its weighted rows by in-range token index
     on top (f16 accumulate; garbage row TRANGE absorbs pad slots)
  3. per-range f16 ReduceScatter overlaps trailing expert compute
Host does: routing decision (indices only), gather, reassembly.

v3 vs v2: f16 partials/collective (half the combine traffic, no 68MB zero
pass), shared-before-expert ordering, and exact (unrounded) per-range
capacity with scatter groups split at range boundaries -- cap drops from
2560 to ~2176 rows, saving ~15% of expert-pass matmul work.
"""

import numpy as np
from contextlib import ExitStack

import concourse.bass as bass
import concourse.bacc as bacc
import concourse.tile as tile
import concourse.mybir as mybir
from concourse import bass_isa, bass_utils
from concourse.bass_interp import get_hw_module

B, S, H = 2, 4096, 2048
E, TOP_K = 8, 2
I_EXP, I_SH = 1024, 4096
T = B * S
NCORES = 8
I_SLICE = I_SH // NCORES       # 512

P = 128
KT = H // P                    # 16
ITE = I_EXP // P               # 8
ITS = I_SLICE // P             # 4
TB = 512
NBLK = T // TB                 # 16
HH = 512
NHH = H // HH                  # 4
RSIZES = [2048, 2048, 2048, 1024, 1024]   # per-range token counts (512-mult)
RSTART = [0, 2048, 4096, 6144, 7168]
NRANGE = len(RSIZES)
assert sum(RSIZES) == T and all(s % TB == 0 for s in RSIZES)

F32 = mybir.dt.float32
F16 = mybir.dt.float16
I16 = mybir.dt.int16
ALU = mybir.AluOpType
ACTF = mybir.ActivationFunctionType


def make_calls(pr):
    """Scatter-call plan from exact per-range padded counts.

    Positions [off[r], off[r]+pr[r]) of each core's gathered-token axis hold
    range r's tokens (shorter cores pad with the garbage row). 128-row
    scatter groups that straddle a range boundary are split into one call
    per range; leading rows of a later-range call point at that range's
    garbage row so the in-SBUF source window can stay partition-aligned.
    Returns (cap, calls) with calls = [(group, range, b)] where rows [0, b)
    of the group's window are written (b is the same on every core)."""
    off = np.concatenate([[0], np.cumsum(pr)])
    total = int(off[-1])
    cap = int(np.ceil(total / P) * P)
    calls = []
    for g in range(cap // P):
        lo, hi = g * P, (g + 1) * P
        for r in range(NRANGE):
            s, e = max(lo, int(off[r])), min(hi, int(off[r]) + int(pr[r]))
            if s < e:
                calls.append((g, r, e - lo))
    return cap, calls


def build_kernel(pr, num_devices=NCORES, with_rs=True, debug_out=False):
    """pr: per-range exact padded counts (max over cores, same on all)."""
    cap, calls = make_calls(pr)
    n_groups = cap // P
    nbe = (cap + TB - 1) // TB
    ncols = len(calls) * 8

    nc = bacc.Bacc(
        "TRN2", target_bir_lowering=False, debug=False, enable_asserts=False,
        num_devices=num_devices, num_swdge_queues=4,
    )
    xT16 = nc.dram_tensor("xT16", [KT, P, T], F16, kind="ExternalInput").ap()
    xTe16 = nc.dram_tensor("xTe16", [KT, P, cap], F16, kind="ExternalInput").ap()
    w9g = nc.dram_tensor("w9g", [KT, P, E], F16, kind="ExternalInput").ap()
    wseg = nc.dram_tensor("wseg", [KT, P, 1], F16, kind="ExternalInput").ap()
    w1e = nc.dram_tensor("w1e", [KT, P, I_EXP], F16, kind="ExternalInput").ap()
    w2e = nc.dram_tensor("w2e", [KT, P, I_EXP], F16, kind="ExternalInput").ap()
    w3e = nc.dram_tensor("w3e", [ITE, P, H], F16, kind="ExternalInput").ap()
    w1s = nc.dram_tensor("w1s", [KT, P, I_SLICE], F16, kind="ExternalInput").ap()
    w2s = nc.dram_tensor("w2s", [KT, P, I_SLICE], F16, kind="ExternalInput").ap()
    w3s = nc.dram_tensor("w3s", [ITS, P, H], F16, kind="ExternalInput").ap()
    sel8d = nc.dram_tensor("sel8", [E, 1], F32, kind="ExternalInput").ap()
    idx16d = nc.dram_tensor("idx16", [P, ncols], I16, kind="ExternalInput").ap()
    out_shard = nc.dram_tensor(
        "out_shard", [T // NCORES, H], F16, kind="ExternalOutput"
    ).ap()

    with tile.TileContext(nc) as tc, ExitStack() as ctx:
        dram = ctx.enter_context(tc.tile_pool(name="dram", bufs=1, space="DRAM"))
        # rows [RSIZES[r], RSIZES[r]+P) are a garbage region for pad slots
        partials = [
            dram.tile([RSIZES[r] + P, H], F16, tag=f"partial{r}",
                      name=f"partial{r}")
            for r in range(NRANGE)
        ]

        # expert-pass consts that should land before the pass starts; the
        # DMAs are emitted inside _shared_pass after its own critical loads
        cstP = ctx.enter_context(tc.tile_pool(name="cstP", bufs=1))
        w9sb = cstP.tile([P, KT, E], F16, tag="w9sb")
        sel8sb = cstP.tile([E, 1], F32, tag="sel8sb")
        idxsb = cstP.tile([P, ncols], I16, tag="idxsb")
        w1esb = cstP.tile([P, KT, I_EXP], F16, tag="w1sb")
        xbe0 = cstP.tile([P, KT, TB], F16, tag="xbe0")
        preload = [
            (w9sb, w9g.rearrange("k p n -> p k n")),
            (sel8sb, sel8d),
            (idxsb, idx16d),
            (w1esb, w1e.rearrange("k p n -> p k n")),
            (xbe0, xTe16[:, :, 0:TB].rearrange("k p t -> p k t")),
        ]

        _shared_pass(nc, tc, partials, xT16, wseg, w1s, w2s, w3s, preload)
        _expert_pass(nc, tc, partials, xTe16, w2e, w3e,
                     cap, nbe, calls, w9sb, sel8sb, idxsb, w1esb, xbe0)

        # ---- per-range ReduceScatter + output -----------------------------
        with tc.tile_pool(name="dramR", bufs=1, space="DRAM") as dramR:
            if with_rs:
                for r in range(NRANGE):
                    rows = RSIZES[r] // NCORES
                    rs_out = dramR.tile(
                        [rows, H], F16, tag=f"rsout{r}", name=f"rsout{r}"
                    )
                    nc.gpsimd.collective_compute(
                        "ReduceScatter",
                        ALU.add,
                        replica_groups=[list(range(NCORES))],
                        ins=[partials[r][0:RSIZES[r], :].opt()],
                        outs=[rs_out.opt()],
                    )
                    o0 = RSTART[r] // NCORES
                    nc.sync.dma_start(out_shard[o0 : o0 + rows], rs_out[:])
            elif debug_out:
                dbg = nc.dram_tensor(
                    "partial_dbg", [T, H], F16, kind="ExternalOutput"
                ).ap()
                for r in range(NRANGE):
                    nc.sync.dma_start(
                        dbg[RSTART[r] : RSTART[r] + RSIZES[r], :],
                        partials[r][0:RSIZES[r], :],
                    )
            else:
                for r in range(NRANGE):
                    o0 = RSTART[r] // NCORES
                    rows = RSIZES[r] // NCORES
                    nc.sync.dma_start(
                        out_shard[o0 : o0 + rows],
                        partials[r][0:rows, :],
                    )

    nc.compile()
    return nc


def _shared_pass(nc, tc, partials, xT16, wseg, w1s, w2s, w3s, preload):
    with (
        tc.tile_pool(name="cstS", bufs=1) as cst,
        tc.tile_pool(name="sbXS", bufs=2) as sbX,
        tc.tile_pool(name="sbHS", bufs=2) as sbH,
        tc.tile_pool(name="sbTS", bufs=3) as sbT,
        tc.tile_pool(name="sbYS", bufs=2) as sbY,
        tc.tile_pool(name="psS", bufs=2, space="PSUM") as ps,
    ):
        # load order: tiny gate vec, block-0 activations + gate/up weights
        # (needed immediately), THEN the down weights (needed ~35us in) --
        # the cost model serializes DMA in issue order, so anything queued
        # ahead of the first matmul's operands delays the whole pass
        wgsb = cst.tile([P, KT, 1], F16, tag="wgsb")
        nc.sync.dma_start(wgsb[:], wseg.rearrange("k p n -> p k n"))
        w1sb = cst.tile([P, KT, I_SLICE], F16, tag="w1sbs")
        nc.sync.dma_start(w1sb[:], w1s.rearrange("k p n -> p k n"))
        xb0 = sbX.tile([P, KT, TB], F16, tag="xbs")
        nc.sync.dma_start(xb0[:], xT16[:, :, 0:TB].rearrange("k p t -> p k t"))
        w2sb = cst.tile([P, KT, I_SLICE], F16, tag="w2sbs")
        nc.sync.dma_start(w2sb[:], w2s.rearrange("k p n -> p k n"))
        w3sb = cst.tile([P, ITS, H], F16, tag="w3sb")
        nc.sync.dma_start(w3sb[:], w3s.rearrange("i p h -> p i h"))
        # expert-pass preloads ride the idle DMA window behind block 0/1
        for tile_, src in preload:
            nc.sync.dma_start(tile_[:], src)

        blk_rng = []
        for r in range(NRANGE):
            blk_rng += [r] * (RSIZES[r] // TB)
        for b in range(NBLK):
            bsl = slice(b * TB, (b + 1) * TB)
            rng_i = blk_rng[b]
            if b == 0:
                xb = xb0
            else:
                xb = sbX.tile([P, KT, TB], F16, tag="xbs")
                nc.sync.dma_start(
                    xb[:], xT16[:, :, bsl].rearrange("k p t -> p k t")
                )

            # sigmoid-gate dot product on DVE (PE is the bottleneck engine)
            acc = sbT.tile([P, TB], F32, tag="gacc")
            tmp = sbT.tile([P, TB], F32, tag="gtmp")
            for ko in range(KT):
                dst = acc if ko == 0 else tmp
                nc.vector.tensor_tensor(
                    dst[:], xb[:, ko],
                    wgsb[:, ko, 0:1].to_broadcast([P, TB]), ALU.mult,
                )
                if ko > 0:
                    nc.vector.tensor_add(out=acc[:], in0=acc[:], in1=tmp[:])
            gsum = sbT.tile([P, TB], F32, tag="gsum")
            nc.gpsimd.partition_all_reduce(
                gsum[:], acc[:], channels=P, reduce_op=bass_isa.ReduceOp.add
            )
            bc_s = sbT.tile([P, TB], F32, tag="bc_s")
            nc.scalar.activation(bc_s[:], gsum[:], ACTF.Sigmoid)

            hc = sbH.tile([P, ITS, TB], F16, tag="hcs")
            for it in range(ITS):
                isl = slice(it * P, (it + 1) * P)
                psG = ps.tile([P, TB], F32, tag="psGs")
                psU = ps.tile([P, TB], F32, tag="psUs")
                for ko in range(KT):
                    nc.tensor.matmul(
                        psG[:], w1sb[:, ko, isl], xb[:, ko],
                        start=(ko == 0), stop=(ko == KT - 1),
                    )
                for ko in range(KT):
                    nc.tensor.matmul(
                        psU[:], w2sb[:, ko, isl], xb[:, ko],
                        start=(ko == 0), stop=(ko == KT - 1),
                    )
                sg = sbT.tile([P, TB], F32, tag="sgs")
                nc.scalar.activation(sg[:], psG[:], ACTF.Silu)
                nc.vector.tensor_mul(out=sg[:], in0=sg[:], in1=psU[:])
                nc.vector.tensor_tensor(hc[:, it], sg[:], bc_s[:], ALU.mult)

            # stage the block's [TB, H] in SBUF f16, then one plain write DMA
            yb = sbY.tile([P, TB // P, H], F16, tag="yblk")
            for hh in range(NHH):
                for ts_ in range(TB // P):
                    psY = ps.tile([P, HH], F32, tag="psYs")
                    for it in range(ITS):
                        nc.tensor.matmul(
                            psY[:],
                            hc[:, it, ts_ * P : (ts_ + 1) * P],
                            w3sb[:, it, hh * HH : (hh + 1) * HH],
                            start=(it == 0), stop=(it == ITS - 1),
                        )
                    nc.vector.tensor_copy(
                        yb[:, ts_, hh * HH : (hh + 1) * HH], psY[:]
                    )
            row0 = b * TB - RSTART[rng_i]
            nc.sync.dma_start(
                partials[rng_i][row0 : row0 + TB, :].rearrange(
                    "(a p) h -> p a h", p=P
                ),
                yb[:],
            )


def _expert_pass(nc, tc, partials, xTe16, w2e, w3e,
                 cap, nbe, calls, w9sb, sel8sb, idxsb, w1sb, xbe0):
    # group -> list of calls for quick lookup. A split (range-boundary)
    # group issues two scatter calls; the ucode may clobber source
    # partitions past a call's count, so the second call reads a duplicate
    # copy of the group's rows staged in an extra yf slot.
    calls_by_block = {}
    extra_per_block = {}
    for ci, (g, r, bcnt) in enumerate(calls):
        eb = g * P // TB
        lst = calls_by_block.setdefault(eb, [])
        ts_ = (g * P - eb * TB) // P
        if any(e[1] == g for e in lst):
            ex = extra_per_block.setdefault(eb, [])
            slot = TB // P + len(ex)
            ex.append((ts_, slot))
        else:
            slot = ts_
        lst.append((ci, g, r, bcnt, slot))
    max_extra = max((len(v) for v in extra_per_block.values()), default=0)

    with (
        tc.tile_pool(name="cstE", bufs=1) as cst,
        tc.tile_pool(name="sbXE", bufs=2) as sbX,
        tc.tile_pool(name="sbHE", bufs=1) as sbH,
        tc.tile_pool(name="sbTE", bufs=2) as sbT,
        tc.tile_pool(name="sbYE", bufs=1) as sbY,
        tc.tile_pool(name="psE", bufs=2, space="PSUM") as ps,
    ):
        # w9/sel/idx/w1/xbe0 preloaded during the shared pass
        xb0 = xbe0
        w2sb = cst.tile([P, KT, I_EXP], F16, tag="w2sb")
        nc.sync.dma_start(w2sb[:], w2e.rearrange("k p n -> p k n"))
        w3esb = cst.tile([P, ITE, H], F16, tag="w3esb")
        nc.sync.dma_start(w3esb[:], w3e.rearrange("i p h -> p i h"))

        for eb in range(nbe):
            tbb = min(TB, cap - eb * TB)
            bsl = slice(eb * TB, eb * TB + tbb)
            if eb == 0:
                xb = xb0
            else:
                xb = sbX.tile([P, KT, TB], F16, tag="xbe")
                nc.sync.dma_start(
                    xb[:, :, 0:tbb], xTe16[:, :, bsl].rearrange("k p t -> p k t")
                )

            # recompute this expert's routing weight for gathered tokens:
            # comb = softmax(logits)[e]  (token is in top-2 by construction)
            ps_l = ps.tile([E, TB], F32, tag="ps_l")
            for ko in range(KT):
                nc.tensor.matmul(
                    ps_l[:, 0:tbb], w9sb[:, ko, :], xb[:, ko, 0:tbb],
                    start=(ko == 0), stop=(ko == KT - 1),
                )
            ex = sbT.tile([E, TB], F32, tag="ex")
            nc.scalar.activation(ex[:, 0:tbb], ps_l[:, 0:tbb], ACTF.Exp)
            sm = sbT.tile([E, TB], F32, tag="sm")
            nc.gpsimd.partition_all_reduce(
                sm[:, 0:tbb], ex[:, 0:tbb], channels=E,
                reduce_op=bass_isa.ReduceOp.add,
            )
            rc = sbT.tile([E, TB], F32, tag="rc")
            nc.vector.reciprocal(rc[:, 0:tbb], sm[:, 0:tbb])
            pr_ = sbT.tile([E, TB], F32, tag="pr")
            nc.vector.tensor_mul(out=pr_[:, 0:tbb], in0=ex[:, 0:tbb],
                                 in1=rc[:, 0:tbb])
            nc.vector.tensor_tensor(
                pr_[:, 0:tbb], pr_[:, 0:tbb],
                sel8sb[:, 0:1].to_broadcast([E, tbb]), ALU.mult
            )
            ce = sbT.tile([E, TB], F32, tag="ce")
            nc.gpsimd.partition_all_reduce(
                ce[:, 0:tbb], pr_[:, 0:tbb], channels=E,
                reduce_op=bass_isa.ReduceOp.add,
            )
            bc_e = sbT.tile([P, TB], F32, tag="bc_e")
            nc.gpsimd.partition_broadcast(
                bc_e[:, 0:tbb], ce[0:1, 0:tbb], channels=P
            )

            hc = sbH.tile([P, ITE, TB], F16, tag="hce")
            for it in range(ITE):
                isl = slice(it * P, (it + 1) * P)
                psG = ps.tile([P, TB], F32, tag="psG")
                psU = ps.tile([P, TB], F32, tag="psU")
                for ko in range(KT):
                    nc.tensor.matmul(
                        psG[:, 0:tbb], w1sb[:, ko, isl], xb[:, ko, 0:tbb],
                        start=(ko == 0), stop=(ko == KT - 1),
                    )
                for ko in range(KT):
                    nc.tensor.matmul(
                        psU[:, 0:tbb], w2sb[:, ko, isl], xb[:, ko, 0:tbb],
                        start=(ko == 0), stop=(ko == KT - 1),
                    )
                sg = sbT.tile([P, TB], F32, tag="sg")
                nc.scalar.activation(sg[:, 0:tbb], psG[:, 0:tbb], ACTF.Silu)
                nc.vector.tensor_mul(out=sg[:, 0:tbb], in0=sg[:, 0:tbb],
                                     in1=psU[:, 0:tbb])
                nc.vector.tensor_tensor(hc[:, it, 0:tbb], sg[:, 0:tbb],
                                        bc_e[:, 0:tbb], ALU.mult)

            # token-major down-proj: yf[tok, h], f16 for the f16 scatter
            yf = sbY.tile([P, TB // P + max_extra, NHH * HH], F16, tag="yf")
            for hh in range(NHH):
                for ts_ in range(tbb // P):
                    psY = ps.tile([P, HH], F32, tag="psY")
                    for it in range(ITE):
                        nc.tensor.matmul(
                            psY[:],
                            hc[:, it, ts_ * P : (ts_ + 1) * P],
                            w3esb[:, it, hh * HH : (hh + 1) * HH],
                            start=(it == 0), stop=(it == ITE - 1),
                        )
                    nc.vector.tensor_copy(
                        yf[:, ts_, hh * HH : (hh + 1) * HH], psY[:]
                    )
            # duplicate boundary-group rows into their extra slots
            for src_ts, slot in extra_per_block.get(eb, []):
                nc.vector.tensor_copy(yf[:, slot, :], yf[:, src_ts, :])
            # scatter-add each group's rows into its range's partial;
            # boundary groups issue one call per overlapped range
            for ci, g, r, bcnt, slot in calls_by_block.get(eb, []):
                nc.gpsimd.dma_scatter_add(
                    out_ap=partials[r][:, :],
                    in_ap=yf[:, slot : slot + 1, :],
                    idxs_ap=idxsb[:, ci * 8 : (ci + 1) * 8],
                    num_idxs=P,
                    num_idxs_reg=int(bcnt),
                    elem_size=H,
                    queue_num=ci % 4,
                )


def route_host(inputs):
    """Routing decision only (indices); all values are recomputed on device."""
    x = np.asarray(inputs["hidden_states"], np.float64).reshape(T, H)
    gw = np.asarray(inputs["gate_w"], np.float64)
    logits = x @ gw.T
    p = np.exp(logits - logits.max(-1, keepdims=True))
    p /= p.sum(-1, keepdims=True)
    order = np.argsort(-p, axis=-1, kind="stable")
    top2 = order[:, :TOP_K]
    per_core = []
    for e in range(NCORES):
        toks = np.where((top2 == e).any(-1))[0]
        per_range = [
            toks[(toks >= RSTART[r]) & (toks < RSTART[r] + RSIZES[r])]
            - RSTART[r]
            for r in range(NRANGE)
        ]
        per_core.append(per_range)
    # per-range padded counts: exact max over cores (no rounding)
    pr = [
        max(len(per_core[c][r]) for c in range(NCORES))
        for r in range(NRANGE)
    ]
    return per_core, pr


def build_idx_and_gather(per_range_tok, pr, cap, calls):
    """Per-core: position->row values (range-relative; pad slots use the
    garbage row TRANGE), per-call wrapped index columns, and absolute gather
    columns (pads gather token 0)."""
    off = np.concatenate([[0], np.cumsum(pr)]).astype(int)
    pos_val = np.full(cap, -1, np.int64)
    gcols = np.zeros(cap, np.int64)
    for r in range(NRANGE):
        toks = per_range_tok[r]
        base = off[r]
        pos_val[base : base + pr[r]] = RSIZES[r]       # garbage default
        pos_val[base : base + len(toks)] = toks
        gcols[base : base + len(toks)] = toks + RSTART[r]
    idx16 = np.zeros((P, len(calls) * 8), np.int16)
    for ci, (g, r, bcnt) in enumerate(calls):
        lo = g * P
        s = max(lo, int(off[r]))
        a = s - lo
        w = np.full(P, -1, np.int16)
        w[0:a] = RSIZES[r]                             # filler -> garbage
        w[a:bcnt] = pos_val[lo + a : lo + bcnt].astype(np.int16)
        mat = w.reshape(8, 16).T
        for k in range(8):
            idx16[k * 16 : (k + 1) * 16, ci * 8 : (ci + 1) * 8] = mat
    return gcols, idx16


def make_in_maps(inputs):
    hs = np.ascontiguousarray(inputs["hidden_states"], dtype=np.float32)
    x = hs.reshape(T, H)
    xT16_flat = np.ascontiguousarray(x.T).astype(np.float16)
    xT16 = xT16_flat.reshape(KT, P, T)

    gate_w = np.asarray(inputs["gate_w"], np.float32)
    seg_w = np.asarray(inputs["shared_expert_gate_w"], np.float32)
    w9g = np.ascontiguousarray(gate_w.T).astype(np.float16).reshape(KT, P, E)
    wseg = np.ascontiguousarray(seg_w.T).astype(np.float16).reshape(KT, P, 1)

    egw = np.asarray(inputs["expert_gate_w"], np.float32)
    euw = np.asarray(inputs["expert_up_w"], np.float32)
    edw = np.asarray(inputs["expert_down_w"], np.float32)
    sgw = np.asarray(inputs["shared_gate_w"], np.float32)
    suw = np.asarray(inputs["shared_up_w"], np.float32)
    sdw = np.asarray(inputs["shared_down_w"], np.float32)

    per_core, pr = route_host(inputs)
    cap, calls = make_calls(pr)

    in_maps = []
    for c in range(NCORES):
        ssl = slice(c * I_SLICE, (c + 1) * I_SLICE)
        gcols, idx16 = build_idx_and_gather(per_core[c], pr, cap, calls)
        xTe16 = np.ascontiguousarray(xT16_flat[:, gcols]).reshape(KT, P, cap)
        sel8 = np.zeros((E, 1), np.float32)
        sel8[c, 0] = 1.0
        in_maps.append(
            {
                "xT16": xT16,
                "xTe16": xTe16,
                "w9g": w9g,
                "wseg": wseg,
                "w1e": np.ascontiguousarray(egw[c].T).astype(np.float16).reshape(KT, P, I_EXP),
                "w2e": np.ascontiguousarray(euw[c].T).astype(np.float16).reshape(KT, P, I_EXP),
                "w3e": np.ascontiguousarray(edw[c].T).astype(np.float16).reshape(ITE, P, H),
                "w1s": np.ascontiguousarray(sgw[ssl].T).astype(np.float16).reshape(KT, P, I_SLICE),
                "w2s": np.ascontiguousarray(suw[ssl].T).astype(np.float16).reshape(KT, P, I_SLICE),
                "w3s": np.ascontiguousarray(sdw[:, ssl].T).astype(np.float16).reshape(ITS, P, H),
                "sel8": sel8,
                "idx16": idx16,
            }
        )
    return in_maps, pr


def assemble_output(results):
    out = np.empty((T, H), np.float32)
    for c in range(NCORES):
        sh = np.asarray(results[c]["out_shard"], np.float32)
        for r in range(NRANGE):
            rows = RSIZES[r] // NCORES
            o0 = RSTART[r] // NCORES
            base = RSTART[r] + c * rows
            out[base : base + rows] = sh[o0 : o0 + rows]
    return out.reshape(B, S, H)


_nc_cache = {}


def kernel(**inputs) -> np.ndarray:
    in_maps, pr = make_in_maps(inputs)
    key = tuple(pr)
    if key not in _nc_cache:
        nc = build_kernel(pr)
        nc.m = get_hw_module(nc.m)
        _nc_cache[key] = nc
    nc = _nc_cache[key]
    res = bass_utils.run_bass_kernel_spmd(
        nc, in_maps, core_ids=list(range(NCORES))
    )
    return assemble_output(res.results)


# revision 24
# speedup vs baseline: 1.3227x; 1.0172x over previous
"""v3: sparse expert-parallel MoE on 8 trn2 cores.

Core c computes: its expert's FFN on only the tokens routed to it
(host-gathered, padded to the per-range max over cores, unrounded), with
routing weights recomputed on device from the gathered activations; plus a
1/8 tensor-parallel slice of the shared expert over all tokens, gated by an
on-device sigmoid.

Output combine, token-major, one f16 partial [TRANGE, H] per token range:
  1. shared pass runs FIRST and writes its dense [TB, H] block results
     directly (plain DMA, no zeroing, no read-modify-write)
  2. expert pass dma_scatter_add's its weighted rows by in-range token index
     on top (f16 accumulate; garbage row TRANGE absorbs pad slots)
  3. per-range f16 ReduceScatter overlaps trailing expert compute
Host does: routing decision (indices only), gather, reassembly.

v3 vs v2: f16 partials/collective (half the combine traffic, no 68MB zero
pass), shared-before-expert ordering, and exact (unrounded) per-range
capacity with scatter groups split at range boundaries -- cap drops from
2560 to ~2176 rows, saving ~15% of expert-pass matmul work.
"""

import numpy as np
from contextlib import ExitStack

import concourse.bass as bass
import concourse.bacc as bacc
import concourse.tile as tile
import concourse.mybir as mybir
from concourse import bass_isa, bass_utils
from concourse.bass_interp import get_hw_module

B, S, H = 2, 4096, 2048
E, TOP_K = 8, 2
I_EXP, I_SH = 1024, 4096
T = B * S
NCORES = 8
I_SLICE = I_SH // NCORES       # 512

P = 128
KT = H // P                    # 16
ITE = I_EXP // P               # 8
ITS = I_SLICE // P             # 4
TB = 512
NBLK = T // TB                 # 16
HH = 512
NHH = H // HH                  # 4
RSIZES = [4096, 2048, 1024, 512, 512]     # per-range token counts (512-mult)
RSTART = [0, 4096, 6144, 7168, 7680]
NRANGE = len(RSIZES)
assert sum(RSIZES) == T and all(s % TB == 0 for s in RSIZES)

F32 = mybir.dt.float32
F16 = mybir.dt.float16
I16 = mybir.dt.int16
ALU = mybir.AluOpType
ACTF = mybir.ActivationFunctionType


def make_calls(pr):
    """Scatter-call plan from exact per-range padded counts.

    Positions [off[r], off[r]+pr[r]) of each core's gathered-token axis hold
    range r's tokens (shorter cores pad with the garbage row). 128-row
    scatter groups that straddle a range boundary are split into one call
    per range; leading rows of a later-range call point at that range's
    garbage row so the in-SBUF source window can stay partition-aligned.
    Returns (cap, calls) with calls = [(group, range, b)] where rows [0, b)
    of the group's window are written (b is the same on every core)."""
    off = np.concatenate([[0], np.cumsum(pr)])
    total = int(off[-1])
    cap = int(np.ceil(total / P) * P)
    calls = []
    for g in range(cap // P):
        lo, hi = g * P, (g + 1) * P
        for r in range(NRANGE):
            s, e = max(lo, int(off[r])), min(hi, int(off[r]) + int(pr[r]))
            if s < e:
                calls.append((g, r, e - lo))
    return cap, calls


def build_kernel(pr, num_devices=NCORES, with_rs=True, debug_out=False):
    """pr: per-range exact padded counts (max over cores, same on all)."""
    cap, calls = make_calls(pr)
    n_groups = cap // P
    nbe = (cap + TB - 1) // TB
    ncols = len(calls) * 8

    nc = bacc.Bacc(
        "TRN2", target_bir_lowering=False, debug=False, enable_asserts=False,
        num_devices=num_devices, num_swdge_queues=4,
    )
    xT16 = nc.dram_tensor("xT16", [KT, P, T], F16, kind="ExternalInput").ap()
    xTe16 = nc.dram_tensor("xTe16", [KT, P, cap], F16, kind="ExternalInput").ap()
    w9g = nc.dram_tensor("w9g", [KT, P, E], F16, kind="ExternalInput").ap()
    wseg = nc.dram_tensor("wseg", [KT, P, 1], F16, kind="ExternalInput").ap()
    w1e = nc.dram_tensor("w1e", [KT, P, I_EXP], F16, kind="ExternalInput").ap()
    w2e = nc.dram_tensor("w2e", [KT, P, I_EXP], F16, kind="ExternalInput").ap()
    w3e = nc.dram_tensor("w3e", [ITE, P, H], F16, kind="ExternalInput").ap()
    w1s = nc.dram_tensor("w1s", [KT, P, I_SLICE], F16, kind="ExternalInput").ap()
    w2s = nc.dram_tensor("w2s", [KT, P, I_SLICE], F16, kind="ExternalInput").ap()
    w3s = nc.dram_tensor("w3s", [ITS, P, H], F16, kind="ExternalInput").ap()
    sel8d = nc.dram_tensor("sel8", [E, 1], F32, kind="ExternalInput").ap()
    idx16d = nc.dram_tensor("idx16", [P, ncols], I16, kind="ExternalInput").ap()
    out_shard = nc.dram_tensor(
        "out_shard", [T // NCORES, H], F16, kind="ExternalOutput"
    ).ap()

    with tile.TileContext(nc) as tc, ExitStack() as ctx:
        dram = ctx.enter_context(tc.tile_pool(name="dram", bufs=1, space="DRAM"))
        # rows [RSIZES[r], RSIZES[r]+P) are a garbage region for pad slots
        partials = [
            dram.tile([RSIZES[r] + P, H], F16, tag=f"partial{r}",
                      name=f"partial{r}")
            for r in range(NRANGE)
        ]

        # expert-pass consts that should land before the pass starts; the
        # DMAs are emitted inside _shared_pass after its own critical loads
        cstP = ctx.enter_context(tc.tile_pool(name="cstP", bufs=1))
        w9sb = cstP.tile([P, KT, E], F16, tag="w9sb")
        sel8sb = cstP.tile([E, 1], F32, tag="sel8sb")
        idxsb = cstP.tile([P, ncols], I16, tag="idxsb")
        w1esb = cstP.tile([P, KT, I_EXP], F16, tag="w1sb")
        xbe0 = cstP.tile([P, KT, TB], F16, tag="xbe0")
        preload = [
            (w9sb, w9g.rearrange("k p n -> p k n")),
            (sel8sb, sel8d),
            (idxsb, idx16d),
            (w1esb, w1e.rearrange("k p n -> p k n")),
            (xbe0, xTe16[:, :, 0:TB].rearrange("k p t -> p k t")),
        ]

        _shared_pass(nc, tc, partials, xT16, wseg, w1s, w2s, w3s, preload)
        _expert_pass(nc, tc, partials, xTe16, w2e, w3e,
                     cap, nbe, calls, w9sb, sel8sb, idxsb, w1esb, xbe0)

        # ---- per-range ReduceScatter + output -----------------------------
        with tc.tile_pool(name="dramR", bufs=1, space="DRAM") as dramR:
            if with_rs:
                for r in range(NRANGE):
                    rows = RSIZES[r] // NCORES
                    rs_out = dramR.tile(
                        [rows, H], F16, tag=f"rsout{r}", name=f"rsout{r}"
                    )
                    nc.gpsimd.collective_compute(
                        "ReduceScatter",
                        ALU.add,
                        replica_groups=[list(range(NCORES))],
                        ins=[partials[r][0:RSIZES[r], :].opt()],
                        outs=[rs_out.opt()],
                    )
                    o0 = RSTART[r] // NCORES
                    nc.sync.dma_start(out_shard[o0 : o0 + rows], rs_out[:])
            elif debug_out:
                dbg = nc.dram_tensor(
                    "partial_dbg", [T, H], F16, kind="ExternalOutput"
                ).ap()
                for r in range(NRANGE):
                    nc.sync.dma_start(
                        dbg[RSTART[r] : RSTART[r] + RSIZES[r], :],
                        partials[r][0:RSIZES[r], :],
                    )
            else:
                for r in range(NRANGE):
                    o0 = RSTART[r] // NCORES
                    rows = RSIZES[r] // NCORES
                    nc.sync.dma_start(
                        out_shard[o0 : o0 + rows],
                        partials[r][0:rows, :],
                    )

    nc.compile()
    return nc


def _shared_pass(nc, tc, partials, xT16, wseg, w1s, w2s, w3s, preload):
    with (
        tc.tile_pool(name="cstS", bufs=1) as cst,
        tc.tile_pool(name="sbXS", bufs=2) as sbX,
        tc.tile_pool(name="sbHS", bufs=2) as sbH,
        tc.tile_pool(name="sbTS", bufs=3) as sbT,
        tc.tile_pool(name="sbYS", bufs=2) as sbY,
        tc.tile_pool(name="psS", bufs=2, space="PSUM") as ps,
    ):
        # load order: tiny gate vec, block-0 activations + gate/up weights
        # (needed immediately), THEN the down weights (needed ~35us in) --
        # the cost model serializes DMA in issue order, so anything queued
        # ahead of the first matmul's operands delays the whole pass
        wgsb = cst.tile([P, KT, 1], F16, tag="wgsb")
        nc.sync.dma_start(wgsb[:], wseg.rearrange("k p n -> p k n"))
        w1sb = cst.tile([P, KT, I_SLICE], F16, tag="w1sbs")
        nc.sync.dma_start(w1sb[:], w1s.rearrange("k p n -> p k n"))
        xb0 = sbX.tile([P, KT, TB], F16, tag="xbs")
        nc.sync.dma_start(xb0[:], xT16[:, :, 0:TB].rearrange("k p t -> p k t"))
        w2sb = cst.tile([P, KT, I_SLICE], F16, tag="w2sbs")
        nc.sync.dma_start(w2sb[:], w2s.rearrange("k p n -> p k n"))
        w3sb = cst.tile([P, ITS, H], F16, tag="w3sb")
        nc.sync.dma_start(w3sb[:], w3s.rearrange("i p h -> p i h"))
        # expert-pass preloads ride the idle DMA window behind block 0/1
        for tile_, src in preload:
            nc.sync.dma_start(tile_[:], src)

        blk_rng = []
        for r in range(NRANGE):
            blk_rng += [r] * (RSIZES[r] // TB)
        for b in range(NBLK):
            bsl = slice(b * TB, (b + 1) * TB)
            rng_i = blk_rng[b]
            if b == 0:
                xb = xb0
            else:
                xb = sbX.tile([P, KT, TB], F16, tag="xbs")
                nc.sync.dma_start(
                    xb[:], xT16[:, :, bsl].rearrange("k p t -> p k t")
                )

            # sigmoid-gate dot product on DVE (PE is the bottleneck engine)
            acc = sbT.tile([P, TB], F32, tag="gacc")
            tmp = sbT.tile([P, TB], F32, tag="gtmp")
            for ko in range(KT):
                dst = acc if ko == 0 else tmp
                nc.vector.tensor_tensor(
                    dst[:], xb[:, ko],
                    wgsb[:, ko, 0:1].to_broadcast([P, TB]), ALU.mult,
                )
                if ko > 0:
                    nc.vector.tensor_add(out=acc[:], in0=acc[:], in1=tmp[:])
            gsum = sbT.tile([P, TB], F32, tag="gsum")
            nc.gpsimd.partition_all_reduce(
                gsum[:], acc[:], channels=P, reduce_op=bass_isa.ReduceOp.add
            )
            bc_s = sbT.tile([P, TB], F32, tag="bc_s")
            nc.scalar.activation(bc_s[:], gsum[:], ACTF.Sigmoid)

            hc = sbH.tile([P, ITS, TB], F16, tag="hcs")
            for it in range(ITS):
                isl = slice(it * P, (it + 1) * P)
                psG = ps.tile([P, TB], F32, tag="psGs")
                psU = ps.tile([P, TB], F32, tag="psUs")
                for ko in range(KT):
                    nc.tensor.matmul(
                        psG[:], w1sb[:, ko, isl], xb[:, ko],
                        start=(ko == 0), stop=(ko == KT - 1),
                    )
                for ko in range(KT):
                    nc.tensor.matmul(
                        psU[:], w2sb[:, ko, isl], xb[:, ko],
                        start=(ko == 0), stop=(ko == KT - 1),
                    )
                sg = sbT.tile([P, TB], F32, tag="sgs")
                nc.scalar.activation(sg[:], psG[:], ACTF.Silu)
                nc.vector.tensor_mul(out=sg[:], in0=sg[:], in1=psU[:])
                nc.vector.tensor_tensor(hc[:, it], sg[:], bc_s[:], ALU.mult)

            # stage the block's [TB, H] in SBUF f16, then one plain write DMA
            yb = sbY.tile([P, TB // P, H], F16, tag="yblk")
            for hh in range(NHH):
                for ts_ in range(TB // P):
                    psY = ps.tile([P, HH], F32, tag="psYs")
                    for it in range(ITS):
                        nc.tensor.matmul(
                            psY[:],
                            hc[:, it, ts_ * P : (ts_ + 1) * P],
                            w3sb[:, it, hh * HH : (hh + 1) * HH],
                            start=(it == 0), stop=(it == ITS - 1),
                        )
                    nc.vector.tensor_copy(
                        yb[:, ts_, hh * HH : (hh + 1) * HH], psY[:]
                    )
            row0 = b * TB - RSTART[rng_i]
            nc.sync.dma_start(
                partials[rng_i][row0 : row0 + TB, :].rearrange(
                    "(a p) h -> p a h", p=P
                ),
                yb[:],
            )


def _expert_pass(nc, tc, partials, xTe16, w2e, w3e,
                 cap, nbe, calls, w9sb, sel8sb, idxsb, w1sb, xbe0):
    # group -> list of calls for quick lookup. A split (range-boundary)
    # group issues two scatter calls; the ucode may clobber source
    # partitions past a call's count, so the second call reads a duplicate
    # copy of the group's rows staged in an extra yf slot.
    calls_by_block = {}
    extra_per_block = {}
    for ci, (g, r, bcnt) in enumerate(calls):
        eb = g * P // TB
        lst = calls_by_block.setdefault(eb, [])
        ts_ = (g * P - eb * TB) // P
        if any(e[1] == g for e in lst):
            ex = extra_per_block.setdefault(eb, [])
            slot = TB // P + len(ex)
            ex.append((ts_, slot))
        else:
            slot = ts_
        lst.append((ci, g, r, bcnt, slot))
    max_extra = max((len(v) for v in extra_per_block.values()), default=0)

    with (
        tc.tile_pool(name="cstE", bufs=1) as cst,
        tc.tile_pool(name="sbXE", bufs=2) as sbX,
        tc.tile_pool(name="sbHE", bufs=1) as sbH,
        tc.tile_pool(name="sbTE", bufs=1) as sbT,
        tc.tile_pool(name="sbYE", bufs=1) as sbY,
        tc.tile_pool(name="psE", bufs=2, space="PSUM") as ps,
    ):
        # w9/sel/idx/w1/xbe0 preloaded during the shared pass
        xb0 = xbe0
        w2sb = cst.tile([P, KT, I_EXP], F16, tag="w2sb")
        nc.sync.dma_start(w2sb[:], w2e.rearrange("k p n -> p k n"))
        w3esb = cst.tile([P, ITE, H], F16, tag="w3esb")
        nc.sync.dma_start(w3esb[:], w3e.rearrange("i p h -> p i h"))

        for eb in range(nbe):
            tbb = min(TB, cap - eb * TB)
            bsl = slice(eb * TB, eb * TB + tbb)
            if eb == 0:
                xb = xb0
            else:
                xb = sbX.tile([P, KT, TB], F16, tag="xbe")
                nc.sync.dma_start(
                    xb[:, :, 0:tbb], xTe16[:, :, bsl].rearrange("k p t -> p k t")
                )

            # recompute this expert's routing weight for gathered tokens:
            # comb = softmax(logits)[e]  (token is in top-2 by construction)
            ps_l = ps.tile([E, TB], F32, tag="ps_l")
            for ko in range(KT):
                nc.tensor.matmul(
                    ps_l[:, 0:tbb], w9sb[:, ko, :], xb[:, ko, 0:tbb],
                    start=(ko == 0), stop=(ko == KT - 1),
                )
            ex = sbT.tile([E, TB], F32, tag="ex")
            nc.scalar.activation(ex[:, 0:tbb], ps_l[:, 0:tbb], ACTF.Exp)
            sm = sbT.tile([E, TB], F32, tag="sm")
            nc.gpsimd.partition_all_reduce(
                sm[:, 0:tbb], ex[:, 0:tbb], channels=E,
                reduce_op=bass_isa.ReduceOp.add,
            )
            rc = sbT.tile([E, TB], F32, tag="rc")
            nc.vector.reciprocal(rc[:, 0:tbb], sm[:, 0:tbb])
            pr_ = sbT.tile([E, TB], F32, tag="pr")
            nc.vector.tensor_mul(out=pr_[:, 0:tbb], in0=ex[:, 0:tbb],
                                 in1=rc[:, 0:tbb])
            nc.vector.tensor_tensor(
                pr_[:, 0:tbb], pr_[:, 0:tbb],
                sel8sb[:, 0:1].to_broadcast([E, tbb]), ALU.mult
            )
            ce = sbT.tile([E, TB], F32, tag="ce")
            nc.gpsimd.partition_all_reduce(
                ce[:, 0:tbb], pr_[:, 0:tbb], channels=E,
                reduce_op=bass_isa.ReduceOp.add,
            )
            bc_e = sbT.tile([P, TB], F32, tag="bc_e")
            nc.gpsimd.partition_broadcast(
                bc_e[:, 0:tbb], ce[0:1, 0:tbb], channels=P
            )

            hc = sbH.tile([P, ITE, TB], F16, tag="hce")
            for it in range(ITE):
                isl = slice(it * P, (it + 1) * P)
                psG = ps.tile([P, TB], F32, tag="psG")
                psU = ps.tile([P, TB], F32, tag="psU")
                for ko in range(KT):
                    nc.tensor.matmul(
                        psG[:, 0:tbb], w1sb[:, ko, isl], xb[:, ko, 0:tbb],
                        start=(ko == 0), stop=(ko == KT - 1),
                    )
                for ko in range(KT):
                    nc.tensor.matmul(
                        psU[:, 0:tbb], w2sb[:, ko, isl], xb[:, ko, 0:tbb],
                        start=(ko == 0), stop=(ko == KT - 1),
                    )
                sg = sbT.tile([P, TB], F32, tag="sg")
                nc.scalar.activation(sg[:, 0:tbb], psG[:, 0:tbb], ACTF.Silu)
                nc.vector.tensor_mul(out=sg[:, 0:tbb], in0=sg[:, 0:tbb],
                                     in1=psU[:, 0:tbb])
                nc.vector.tensor_tensor(hc[:, it, 0:tbb], sg[:, 0:tbb],
                                        bc_e[:, 0:tbb], ALU.mult)

            # token-major down-proj: yf[tok, h], f16 for the f16 scatter
            yf = sbY.tile([P, TB // P + max_extra, NHH * HH], F16, tag="yf")
            for hh in range(NHH):
                for ts_ in range(tbb // P):
                    psY = ps.tile([P, HH], F32, tag="psY")
                    for it in range(ITE):
                        nc.tensor.matmul(
                            psY[:],
                            hc[:, it, ts_ * P : (ts_ + 1) * P],
                            w3esb[:, it, hh * HH : (hh + 1) * HH],
                            start=(it == 0), stop=(it == ITE - 1),
                        )
                    nc.vector.tensor_copy(
                        yf[:, ts_, hh * HH : (hh + 1) * HH], psY[:]
                    )
            # duplicate boundary-group rows into their extra slots
            for src_ts, slot in extra_per_block.get(eb, []):
                nc.vector.tensor_copy(yf[:, slot, :], yf[:, src_ts, :])
            # scatter-add each group's rows into its range's partial;
            # boundary groups issue one call per overlapped range
            for ci, g, r, bcnt, slot in calls_by_block.get(eb, []):
                nc.gpsimd.dma_scatter_add(
                    out_ap=partials[r][:, :],
                    in_ap=yf[:, slot : slot + 1, :],
                    idxs_ap=idxsb[:, ci * 8 : (ci + 1) * 8],
                    num_idxs=P,
                    num_idxs_reg=int(bcnt),
                    elem_size=H,
                    queue_num=ci % 4,
                )


def route_host(inputs):
    """Routing decision only (indices); all values are recomputed on device."""
    x = np.asarray(inputs["hidden_states"], np.float64).reshape(T, H)
    gw = np.asarray(inputs["gate_w"], np.float64)
    logits = x @ gw.T
    p = np.exp(logits - logits.max(-1, keepdims=True))
    p /= p.sum(-1, keepdims=True)
    order = np.argsort(-p, axis=-1, kind="stable")
    top2 = order[:, :TOP_K]
    per_core = []
    for e in range(NCORES):
        toks = np.where((top2 == e).any(-1))[0]
        per_range = [
            toks[(toks >= RSTART[r]) & (toks < RSTART[r] + RSIZES[r])]
            - RSTART[r]
            for r in range(NRANGE)
        ]
        per_core.append(per_range)
    # per-range padded counts: exact max over cores (no rounding)
    pr = [
        max(len(per_core[c][r]) for c in range(NCORES))
        for r in range(NRANGE)
    ]
    return per_core, pr


def build_idx_and_gather(per_range_tok, pr, cap, calls):
    """Per-core: position->row values (range-relative; pad slots use the
    garbage row TRANGE), per-call wrapped index columns, and absolute gather
    columns (pads gather token 0)."""
    off = np.concatenate([[0], np.cumsum(pr)]).astype(int)
    pos_val = np.full(cap, -1, np.int64)
    gcols = np.zeros(cap, np.int64)
    for r in range(NRANGE):
        toks = per_range_tok[r]
        base = off[r]
        pos_val[base : base + pr[r]] = RSIZES[r]       # garbage default
        pos_val[base : base + len(toks)] = toks
        gcols[base : base + len(toks)] = toks + RSTART[r]
    idx16 = np.zeros((P, len(calls) * 8), np.int16)
    for ci, (g, r, bcnt) in enumerate(calls):
        lo = g * P
        s = max(lo, int(off[r]))
        a = s - lo
        w = np.full(P, -1, np.int16)
        w[0:a] = RSIZES[r]                             # filler -> garbage
        w[a:bcnt] = pos_val[lo + a : lo + bcnt].astype(np.int16)
        mat = w.reshape(8, 16).T
        for k in range(8):
            idx16[k * 16 : (k + 1) * 16, ci * 8 : (ci + 1) * 8] = mat
    return gcols, idx16


def make_in_maps(inputs):
    hs = np.ascontiguousarray(inputs["hidden_states"], dtype=np.float32)
    x = hs.reshape(T, H)
    xT16_flat = np.ascontiguousarray(x.T).astype(np.float16)
    xT16 = xT16_flat.reshape(KT, P, T)

    gate_w = np.asarray(inputs["gate_w"], np.float32)
    seg_w = np.asarray(inputs["shared_expert_gate_w"], np.float32)
    w9g = np.ascontiguousarray(gate_w.T).astype(np.float16).reshape(KT, P, E)
    wseg = np.ascontiguousarray(seg_w.T).astype(np.float16).reshape(KT, P, 1)

    egw = np.asarray(inputs["expert_gate_w"], np.float32)
    euw = np.asarray(inputs["expert_up_w"], np.float32)
    edw = np.asarray(inputs["expert_down_w"], np.float32)
    sgw = np.asarray(inputs["shared_gate_w"], np.float32)
    suw = np.asarray(inputs["shared_up_w"], np.float32)
    sdw = np.asarray(inputs["shared_down_w"], np.float32)

    per_core, pr = route_host(inputs)
    cap, calls = make_calls(pr)

    in_maps = []
    for c in range(NCORES):
        ssl = slice(c * I_SLICE, (c + 1) * I_SLICE)
        gcols, idx16 = build_idx_and_gather(per_core[c], pr, cap, calls)
        xTe16 = np.ascontiguousarray(xT16_flat[:, gcols]).reshape(KT, P, cap)
        sel8 = np.zeros((E, 1), np.float32)
        sel8[c, 0] = 1.0
        in_maps.append(
            {
                "xT16": xT16,
                "xTe16": xTe16,
                "w9g": w9g,
                "wseg": wseg,
                "w1e": np.ascontiguousarray(egw[c].T).astype(np.float16).reshape(KT, P, I_EXP),
                "w2e": np.ascontiguousarray(euw[c].T).astype(np.float16).reshape(KT, P, I_EXP),
                "w3e": np.ascontiguousarray(edw[c].T).astype(np.float16).reshape(ITE, P, H),
                "w1s": np.ascontiguousarray(sgw[ssl].T).astype(np.float16).reshape(KT, P, I_SLICE),
                "w2s": np.ascontiguousarray(suw[ssl].T).astype(np.float16).reshape(KT, P, I_SLICE),
                "w3s": np.ascontiguousarray(sdw[:, ssl].T).astype(np.float16).reshape(ITS, P, H),
                "sel8": sel8,
                "idx16": idx16,
            }
        )
    return in_maps, pr


def assemble_output(results):
    out = np.empty((T, H), np.float32)
    for c in range(NCORES):
        sh = np.asarray(results[c]["out_shard"], np.float32)
        for r in range(NRANGE):
            rows = RSIZES[r] // NCORES
            o0 = RSTART[r] // NCORES
            base = RSTART[r] + c * rows
            out[base : base + rows] = sh[o0 : o0 + rows]
    return out.reshape(B, S, H)


_nc_cache = {}


def kernel(**inputs) -> np.ndarray:
    in_maps, pr = make_in_maps(inputs)
    key = tuple(pr)
    if key not in _nc_cache:
        nc = build_kernel(pr)
        nc.m = get_hw_module(nc.m)
        _nc_cache[key] = nc
    nc = _nc_cache[key]
    res = bass_utils.run_bass_kernel_spmd(
        nc, in_maps, core_ids=list(range(NCORES))
    )
    return assemble_output(res.results)
